# revision 51
# baseline (speedup 1.0000x reference)
"""Trainium2 Bass kernel for the gnn_message_passing block (8 NeuronCores).

Strategy (per core c, owning 512 global rows r = c*512..(c+1)*512):
  - Host rotates x_node/x_edge rows by -r0 so the owned rows sit first on
    every core (SPMD: one program, per-core data).
  - Associativity: mat @ (x @ W.T) == (mat @ x) @ W.T, so the five big
    N x N aggregations (adj@h shared by modules 0/4, four proj@k inputs)
    are computed ONCE per core as row-blocks (mat[r].T streamed from DRAM
    as the bf16 moving operand; bf16 h/e tiles stationary), producing
    feature-major f32r outputs that feed the 512x512 projections.
  - rmsnorm weight vectors and the 1/sqrt(D) score scale are folded into
    the projection weights host-side; on-chip rmsnorm is the pure
    x * rsqrt(mean(x^2)+eps) form, computed on ACT (square+accum).
  - Per-node 8-head SDPA runs on DVE in fp16 (2x mode) with broadcast-AP
    multiplies and halving-tree + segmented reduces; exp on ACT.
  - Module processing is interleaved with the aggregations so PE (matmuls)
    and DVE (SDPA) run concurrently:
      loads(h,e0-3) | mod 1,5 | agg0 | mod 0,4 | agg1,2 | loads(e) |
      agg3 | mod 3 | agg4 | mod 7 | mod 2 | FFN-h | mod 6 | FFN-e
  - FFNs: feature-major matmuls, gelu(+bias) on ACT, bias2 via K=1 matmul.
Projection/FFN matmuls use float32r (full-rate fp32, moving dim >= 256).
DMA loads are batched (4-8 tiles per SWDGE start) to keep Pool free.
"""
import numpy as np

N = 4096
E = 512
H = 8
D = 64
FF = 2048
P = 128
NCORES = 8
RPC = N // NCORES  # 512 rows per core
NT = N // P        # 32 tiles over all nodes
LT = RPC // P      # 4 local tiles
EPS = float(np.finfo(np.float32).eps)

_PROGRAM_CACHE = {}


def _split_big_waits(nc, mybir):
    """walrus in this toolchain rejects multi-wait instructions; cap at 1
    (2 for EventSemaphore), chaining the excess as EventSemaphores."""
    for f in nc.m.functions:
        for bb in f.blocks:
            insts = list(bb.instructions)
            out = []
            changed = False
            for inst in insts:
                si = inst.sync_info
                waits = list(si.on_wait) if si and si.on_wait else []
                cap = 2 if isinstance(inst, mybir.InstEventSemaphore) else 1
                if len(waits) > cap:
                    extra, keep = waits[:-cap], waits[-cap:]
                    for ci in range(0, len(extra), 2):
                        ev = mybir.InstEventSemaphore(name=f"{inst.name}-evw{ci}")
                        ev.engine = inst.engine
                        ev.sync_info = mybir.SyncInfo(on_wait=extra[ci:ci + 2],
                                                      on_update=[])
                        out.append(ev)
                    si.on_wait = keep
                    changed = True
                out.append(inst)
            if changed:
                bb.instructions[:] = out


def _build_program():
    import concourse.bass as bass
    import concourse.tile as tile
    from concourse import mybir
    from concourse.masks import make_identity
    from contextlib import ExitStack

    f32 = mybir.dt.float32
    f32r = mybir.dt.float32r
    f16 = mybir.dt.float16
    bf16 = mybir.dt.bfloat16
    AF = mybir.ActivationFunctionType
    OP = mybir.AluOpType
    AX = mybir.AxisListType

    def bc(t, dims, off=0):
        return bass.AP(tensor=t.tensor, offset=t.offset + off,
                       ap=[list(t.ap[0])] + [[s, c] for (s, c) in dims])

    nc = bass.Bass()

    xn_d = nc.declare_dram_parameter("xn", [RPC, E], bf16, isOutput=False)
    xe_d = nc.declare_dram_parameter("xe", [RPC, E], bf16, isOutput=False)
    mat_d = [nc.declare_dram_parameter(f"mat{i}", [N, RPC], bf16, isOutput=False)
             for i in range(5)]
    wq_d = nc.declare_dram_parameter("wqT", [H, E, E], bf16, isOutput=False)
    wk_d = nc.declare_dram_parameter("wkT", [H, E, E], bf16, isOutput=False)
    wv_d = nc.declare_dram_parameter("wvT", [H, E, E], bf16, isOutput=False)
    w1h_d = nc.declare_dram_parameter("w1hT", [E, FF], bf16, isOutput=False)
    w2h_d = nc.declare_dram_parameter("w2hT", [FF, E], bf16, isOutput=False)
    w1e_d = nc.declare_dram_parameter("w1eT", [E, FF], bf16, isOutput=False)
    w2e_d = nc.declare_dram_parameter("w2eT", [FF, E], bf16, isOutput=False)
    b1h_d = nc.declare_dram_parameter("b1h", [FF], f32, isOutput=False)
    b2h_d = nc.declare_dram_parameter("b2h", [E], f32, isOutput=False)
    b1e_d = nc.declare_dram_parameter("b1e", [FF], f32, isOutput=False)
    b2e_d = nc.declare_dram_parameter("b2e", [E], f32, isOutput=False)
    outh_d = nc.declare_dram_parameter("outh", [RPC, E], f32, isOutput=True)
    oute_d = nc.declare_dram_parameter("oute", [RPC, E], f32, isOutput=True)
    # internal DRAM bounce buffers for the normed-x AllGather
    gin_h = nc.dram_tensor("gin_h", [RPC, E], bf16)
    gout_h = nc.dram_tensor("gout_h", [N, E], bf16, addr_space="Shared")
    gin_e = nc.dram_tensor("gin_e", [RPC, E], bf16)
    gout_e = nc.dram_tensor("gout_e", [N, E], bf16, addr_space="Shared")

    with tile.TileContext(nc, pool_alloc_mode="queue") as tc, ExitStack() as ctx:
        consts = ctx.enter_context(tc.tile_pool(name="consts", bufs=1))
        ident = consts.tile([P, P], f32)
        make_identity(nc, ident)
        ident_bf = consts.tile([P, P], bf16)
        nc.scalar.copy(ident_bf[:], ident[:])
        ones1f = consts.tile([1, P], f32)
        nc.gpsimd.memset(ones1f, 1.0)
        ones1 = consts.tile([1, P], f32r)
        nc.scalar.copy(ones1[:], ones1f[:])
        eps_t = consts.tile([P, 1], f32)
        nc.vector.memset(eps_t, EPS)
        b1h_t = consts.tile([P, FF // P], f32)
        nc.gpsimd.dma_start(out=b1h_t, in_=b1h_d[:].rearrange("(c p) -> p c", p=P))
        b1e_t = consts.tile([P, FF // P], f32)
        nc.gpsimd.dma_start(out=b1e_t, in_=b1e_d[:].rearrange("(c p) -> p c", p=P))
        b2h_t = consts.tile([1, E], f32r)
        nc.gpsimd.dma_start(out=b2h_t, in_=b2h_d[:].rearrange("(a e) -> a e", a=1))
        b2e_t = consts.tile([1, E], f32r)
        nc.gpsimd.dma_start(out=b2e_t, in_=b2e_d[:].rearrange("(a e) -> a e", a=1))

        # whole-program pools
        locp = ctx.enter_context(tc.tile_pool(name="loc", bufs=1))
        attp = ctx.enter_context(tc.tile_pool(name="att", bufs=1))
        statp = ctx.enter_context(tc.tile_pool(name="stat", bufs=4))
        sqscp = ctx.enter_context(tc.tile_pool(name="sqsc", bufs=1))
        wpool = ctx.enter_context(tc.tile_pool(name="wts", bufs=1))
        qkvp = ctx.enter_context(tc.tile_pool(name="qkv", bufs=1))
        tmpp = ctx.enter_context(tc.tile_pool(name="sdtmp", bufs=1))
        smp = ctx.enter_context(tc.tile_pool(name="sdsm", bufs=2))
        psp = ctx.enter_context(tc.tile_pool(name="ps", bufs=1, space="PSUM"))

        hTl = [locp.tile([P, RPC], bf16, tag=f"hTl{fc}", name=f"hTl{fc}")
               for fc in range(4)]
        eTl = [locp.tile([P, RPC], bf16, tag=f"eTl{fc}", name=f"eTl{fc}")
               for fc in range(4)]
        xatt_h = [attp.tile([P, E], f32, tag=f"xh{t}", name=f"xh{t}")
                  for t in range(LT)]
        xatt_e = [attp.tile([P, E], f32, tag=f"xe{t}", name=f"xe{t}")
                  for t in range(LT)]

        def load_norm(x_dram, dst_tiles, t0, t1, xpool, on_act=False):
            """Stream x rows [t0*128, t1*128) in 4-tile DMAs; rmsnorm each.
            ACT computes sum-of-squares + Rsqrt; the normalizing multiply
            runs on DVE (fast path feeding transposes) or fully on ACT
            (bulk tiles, keeping the DVE queue free for SDPA)."""
            for g0 in range(t0, t1, 4):
                ng = min(4, t1 - g0)
                xg = xpool.tile([P, ng * E], bf16, tag="xing", name="xing")
                nc.sync.dma_start(
                    out=xg.rearrange("p (t e) -> p t e", e=E),
                    in_=x_dram[g0 * P:(g0 + ng) * P, :].rearrange(
                        "(t p) e -> p t e", p=P))
                ssq = statp.tile([P, 4], f32, tag="ssq", name="ssq")
                sc = sqscp.tile([P, E], f32, tag="sqsc", name="sqsc")
                for t in range(ng):
                    nc.scalar.activation(out=sc[:], in_=xg[:, t * E:(t + 1) * E],
                                         func=AF.Square,
                                         accum_out=ssq[:, t:t + 1])
                # rs = (mean+eps)^-1/2 = exp(-ln(mean+eps)/2): stays in the
                # natural_log_exp ACT table set shared with the SDPA exp
                lnv = statp.tile([P, 4], f32, tag="lnv", name="lnv")
                nc.scalar.activation(out=lnv[:, :ng], in_=ssq[:, :ng],
                                     func=AF.Ln, bias=eps_t[:], scale=1.0 / E)
                rs = statp.tile([P, 4], f32, tag="rs", name="rs")
                nc.scalar.activation(out=rs[:, :ng], in_=lnv[:, :ng],
                                     func=AF.Exp, scale=-0.5)
                for t in range(ng):
                    ti = g0 + t
                    if on_act:
                        nc.scalar.activation(out=dst_tiles[ti][:],
                                             in_=xg[:, t * E:(t + 1) * E],
                                             func=AF.Copy, scale=rs[:, t:t + 1])
                    else:
                        nc.vector.tensor_scalar_mul(
                            dst_tiles[ti][:], xg[:, t * E:(t + 1) * E],
                            rs[:, t:t + 1])

        def rmsnorm_tile(dst, src_ap):
            """dst = pure rmsnorm of node-major [128, 512] slice (FFN path)."""
            sc = sqscp.tile([P, E], f32, tag="sqsc", name="sqsc")
            ssq = statp.tile([P, 1], f32, tag="ssq2", name="ssq2")
            nc.scalar.activation(out=sc[:], in_=src_ap, func=AF.Square,
                                 accum_out=ssq[:])
            lnv = statp.tile([P, 1], f32, tag="lnv2", name="lnv2")
            nc.scalar.activation(out=lnv[:], in_=ssq[:], func=AF.Ln,
                                 bias=eps_t[:], scale=1.0 / E)
            rs = statp.tile([P, 1], f32, tag="rs2", name="rs2")
            nc.scalar.activation(out=rs[:], in_=lnv[:], func=AF.Exp, scale=-0.5)
            nc.vector.tensor_scalar_mul(dst[:], src_ap, rs[:])

        def transpose_local(srcs, dstT):
            for fc in range(4):
                ps = psp.tile([P, RPC], bf16, tag="projps", bufs=2, name="trps")
                for t in range(4):
                    nc.tensor.transpose(ps[:, t * P:(t + 1) * P],
                                        srcs[t][:, fc * P:(fc + 1) * P],
                                        ident_bf[:])
                nc.vector.tensor_copy(dstT[fc][:], ps[:])

        def aggregate(mi, lhs_tiles, aggpool):
            """returns 4 feature-major bf16 [128, 512] blocks of mat_mi @ x."""
            mst = ExitStack()
            matgp = mst.enter_context(tc.tile_pool(name=f"matg{mi}", bufs=2))
            pss = [psp.tile([P, E], f32, tag=f"agps{b}", name=f"agps{b}")
                   for b in range(4)]
            for g in range(8):
                mt = matgp.tile([P, 4 * RPC], bf16, tag="matg", name="matg")
                nc.sync.dma_start(
                    out=mt.rearrange("p (t e) -> p t e", e=RPC),
                    in_=mat_d[mi][g * 4 * P:(g + 1) * 4 * P, :].rearrange(
                        "(t p) e -> p t e", p=P))
                for t in range(4):
                    ti = g * 4 + t
                    for b in range(4):
                        nc.tensor.matmul(
                            pss[b][:],
                            lhsT=lhs_tiles[ti][:, b * P:(b + 1) * P],
                            rhs=mt[:, t * RPC:(t + 1) * RPC],
                            start=(ti == 0), stop=(ti == NT - 1))
            outt = []
            for b in range(4):
                at = aggpool.tile([P, E], bf16, tag=f"ag{mi}_{b}",
                                  name=f"ag{mi}_{b}")
                nc.scalar.copy(at[:], pss[b][:])
                outt.append(at)
            mst.close()
            return outt

        def module(m, qsrc, ksrc, branch_att, first):
            w_ts = {}
            for (dram, nm) in ((wq_d, "wq"), (wk_d, "wk"), (wv_d, "wv")):
                wt = wpool.tile([P, 4 * E], bf16, tag=nm, name=f"w_{nm}")
                nc.scalar.dma_start(
                    out=wt.rearrange("p (fc e) -> p fc e", e=E),
                    in_=dram[m].rearrange("(fc p) e -> p fc e", p=P))
                w_ts[nm] = wt

            # phase 1: per tile, project q/k/v and reduce scores into s_all
            s_all = smp.tile([P, LT * H * H], f16, tag="s", name="s")
            v_ts = []
            for t in range(LT):
                # q and k share one 2-bank psum tile and one ACT copy
                qk_ps = psp.tile([P, 2 * E], f32, tag="projqk", bufs=1,
                                 name="qkps")
                for (srcT, wnm, half) in ((qsrc, "wq", 0), (ksrc, "wk", 1)):
                    wt = w_ts[wnm]
                    for fc in range(4):
                        nc.tensor.matmul(
                            qk_ps[:, half * E:(half + 1) * E],
                            lhsT=srcT[fc][:, t * P:(t + 1) * P],
                            rhs=wt[:, fc * E:(fc + 1) * E],
                            start=(fc == 0), stop=(fc == 3))
                qk_t = qkvp.tile([P, 2 * E], f16, tag=f"qk_{t}", name=f"qk{t}")
                nc.scalar.copy(qk_t[:], qk_ps[:])
                q_t, k_t = qk_t, None  # k read via off=E on qk_t

                ps = psp.tile([P, E], f32, tag="projps", bufs=2, name="projps")
                wt = w_ts["wv"]
                for fc in range(4):
                    nc.tensor.matmul(
                        ps[:],
                        lhsT=hTl[fc][:, t * P:(t + 1) * P],
                        rhs=wt[:, fc * E:(fc + 1) * E],
                        start=(fc == 0), stop=(fc == 3))
                dt = qkvp.tile([P, E], f16, tag=f"wv_{t}", name=f"v{t}")
                # stored d-major, head-minor: dt[d*8+g] = ps[g*64+d]
                nc.scalar.copy(bc(dt, [(1, 8), (8, 64)]), ps[:])
                v_ts.append(dt)
                # scores, (h,g,d) layout: tmp[h*512+g*64+d] = q[h,d]*k[g,d]
                tmp = tmpp.tile([P, H * H * D], f16, tag="sdpa", bufs=2,
                                name="sdpa")
                nc.vector.tensor_tensor(
                    out=bc(tmp, [(512, 8), (64, 8), (1, 64)]),
                    in0=bc(q_t, [(64, 8), (0, 8), (1, 64)]),
                    in1=bc(q_t, [(0, 8), (64, 8), (1, 64)], off=E),
                    op=OP.mult)
                # halving tree over inner d: strided src, contiguous dst
                szs = (2048, 1024)
                bufs = [tmpp.tile([P, n], f16, tag=f"pp{n}", bufs=1,
                                  name=f"str{n}") for n in szs]
                src, run = tmp, 32
                for bi, n in enumerate(szs[:2]):
                    nc.vector.tensor_tensor(
                        out=bc(bufs[bi], [(1, n)]),
                        in0=bc(src, [(2 * run, 64), (1, run)]),
                        in1=bc(src, [(2 * run, 64), (1, run)], off=run),
                        op=OP.add)
                    src, run = bufs[bi], run // 2
                # remaining 16-wide groups in one 1x tensor_reduce
                with nc.allow_low_precision(reason="f16 score partial sums"):
                    nc.vector.tensor_reduce(
                        out=bc(s_all, [(1, 64)], off=t * H * H),
                        in_=bc(src, [(16, 64), (1, 16)]),
                        axis=AX.X, op=OP.add)

            # phase 2: one exp / den / recip for all 4 tiles
            ex_all = smp.tile([P, LT * H * H], f16, tag="ex", name="ex")
            nc.scalar.activation(out=ex_all[:], in_=s_all[:], func=AF.Exp)
            den = smp.tile([P, LT * H], f32, tag="den", name="den")
            nc.vector.tensor_reduce(
                out=den[:], in_=ex_all.rearrange("p (th g) -> p th g", g=H),
                axis=AX.X, op=OP.add)
            rden = smp.tile([P, LT * H], f16, tag="rden", name="rden")
            with nc.allow_low_precision(reason="f16 softmax denominators"):
                nc.vector.reciprocal(out=rden[:], in_=den[:])

            # phase 3: AV per tile with unnormalized ex; (h,d,g), g innermost
            for t in range(LT):
                v_t = v_ts[t]
                tmp2 = tmpp.tile([P, H * H * D], f16, tag="sdpa", bufs=2,
                                 name="sdpa2")
                nc.vector.tensor_tensor(
                    out=bc(tmp2, [(512, 8), (8, 64), (1, 8)]),
                    in0=bc(ex_all, [(8, 8), (0, 64), (1, 8)], off=t * H * H),
                    in1=bc(v_t, [(0, 8), (8, 64), (1, 8)]),
                    op=OP.mult)
                # pair tree over g: +4 (runs-4), +2 (runs-2), +1 (stride-2)
                av1 = tmpp.tile([P, 2048], f16, tag="pp2048", bufs=1, name="av1")
                nc.vector.tensor_tensor(
                    out=bc(av1, [(1, 2048)]),
                    in0=bc(tmp2, [(8, 512), (1, 4)]),
                    in1=bc(tmp2, [(8, 512), (1, 4)], off=4),
                    op=OP.add)
                av2 = tmpp.tile([P, 1024], f16, tag="pp1024", bufs=1, name="av2")
                nc.vector.tensor_tensor(
                    out=bc(av2, [(1, 1024)]),
                    in0=bc(av1, [(4, 512), (1, 2)]),
                    in1=bc(av1, [(4, 512), (1, 2)], off=2),
                    op=OP.add)
                avf = smp.tile([P, E], f16, tag="avf", name="avf")
                nc.vector.tensor_tensor(
                    out=avf[:],
                    in0=bc(av2, [(2, 512)]),
                    in1=bc(av2, [(2, 512)], off=1),
                    op=OP.add)
                # fold 1/den (per (n,h)) while writing the branch accumulator
                if first:
                    nc.vector.tensor_tensor(
                        out=branch_att[t][:], in0=avf[:],
                        in1=bc(rden, [(1, 8), (0, 64)], off=t * H),
                        op=OP.mult)
                else:
                    rt = smp.tile([P, E], f16, tag="avred", name="avred")
                    nc.vector.tensor_tensor(
                        out=rt[:], in0=avf[:],
                        in1=bc(rden, [(1, 8), (0, 64)], off=t * H),
                        op=OP.mult)
                    nc.gpsimd.tensor_tensor(out=branch_att[t][:],
                                            in0=branch_att[t][:], in1=rt[:],
                                            op=OP.add)

        def ffn(branch_att, w1_dram, w2_dram, b1_t, b2_t, out_dram):
            with tc.tile_pool(name="ffn_sb", bufs=1) as fsb, \
                 tc.tile_pool(name="ffn_xn", bufs=1) as fxn:
                xn_tiles = []
                for t in range(LT):
                    xt = fxn.tile([P, E], bf16, tag=f"fx{t}", name=f"fx{t}")
                    rmsnorm_tile(xt, branch_att[t][:])
                    xn_tiles.append(xt)
                xnT = []
                for fc in range(4):
                    ps = psp.tile([P, RPC], bf16, tag="agps0", name="ftr")
                    for t in range(4):
                        nc.tensor.transpose(ps[:, t * P:(t + 1) * P],
                                            xn_tiles[t][:, fc * P:(fc + 1) * P],
                                            ident_bf[:])
                    xt = fxn.tile([P, RPC], bf16, tag=f"fxT{fc}", name=f"fxT{fc}")
                    nc.scalar.copy(xt[:], ps[:])
                    xnT.append(xt)
                g1 = []
                HW1 = FF // 2
                for half in range(2):
                    w1_t = fsb.tile([P, 4 * HW1], bf16, tag="w1", name="w1")
                    nc.scalar.dma_start(
                        out=w1_t.rearrange("p (fc e) -> p fc e", e=HW1),
                        in_=w1_dram[:, half * HW1:(half + 1) * HW1].rearrange(
                            "(fc p) e -> p fc e", p=P))
                    for fb in range(HW1 // P):
                        ffb = half * (HW1 // P) + fb
                        ps = psp.tile([P, RPC], f32, tag=f"agps{1 + ffb % 2}",
                                      name="fps1")
                        for fc in range(4):
                            nc.tensor.matmul(
                                ps[:],
                                lhsT=w1_t[:, fc * HW1 + fb * P:
                                          fc * HW1 + (fb + 1) * P],
                                rhs=xnT[fc][:],
                                start=(fc == 0), stop=(fc == 3))
                        gt = fsb.tile([P, RPC], bf16, tag=f"g1_{ffb}",
                                      name=f"g1_{ffb}")
                        nc.scalar.activation(out=gt[:], in_=ps[:], func=AF.Gelu,
                                             bias=b1_t[:, ffb:ffb + 1], scale=1.0)
                        g1.append(gt)
                w2_t = fsb.tile([P, 16 * E], bf16, tag="w2", name="w2")
                nc.scalar.dma_start(
                    out=w2_t.rearrange("p (fc e) -> p fc e", e=E),
                    in_=w2_dram[:, :].rearrange("(fc p) e -> p fc e", p=P))
                ot = fsb.tile([P, 4 * E], f32, tag="fo", name="fo")
                for b in range(LT):
                    ps = psp.tile([P, E], f32, tag="agps3", name="fps2")
                    for ffc in range(FF // P):
                        nc.tensor.matmul(
                            ps[:],
                            lhsT=g1[ffc][:, b * P:(b + 1) * P],
                            rhs=w2_t[:, ffc * E:(ffc + 1) * E],
                            start=(ffc == 0), stop=False)
                    nc.tensor.matmul(ps[:], lhsT=ones1[:], rhs=b2_t[:],
                                     start=False, stop=True)
                    nc.scalar.copy(ot[:, b * E:(b + 1) * E], ps[:])
                nc.sync.dma_start(
                    out=out_dram[:, :].rearrange("(b p) e -> p b e", p=P),
                    in_=ot.rearrange("p (b e) -> p b e", e=E))

        # ======== emission order (the schedule) ========
        # Pool open/close must be LIFO: agglate (aggs 1-4) and eearly
        # outlive hfull; agg0/erest nest inside.
        aggl12_stack = ExitStack()
        agglate12 = aggl12_stack.enter_context(
            tc.tile_pool(name="agglate12", bufs=1))
        aggl34_stack = ExitStack()
        agglate34 = aggl34_stack.enter_context(
            tc.tile_pool(name="agglate34", bufs=1))
        eearly_stack = ExitStack()
        eearly = eearly_stack.enter_context(tc.tile_pool(name="eearly", bufs=1))
        hstack = ExitStack()
        hfp = hstack.enter_context(tc.tile_pool(name="hfull", bufs=1))

        h_sb = [hfp.tile([P, E], bf16, tag=f"h{t}", name=f"hsb{t}")
                for t in range(NT)]
        e_sb = [eearly.tile([P, E], bf16, tag=f"e{t}", name=f"esb{t}")
                for t in range(NT)]
        h_own = [hfp.tile([P, E], bf16, tag=f"ho{t}", name=f"hown{t}")
                 for t in range(LT)]
        e_own = [eearly.tile([P, E], bf16, tag=f"eo{t}", name=f"eown{t}")
                 for t in range(LT)]

        def gather_x(own, gin, gout, full):
            """DMA own normed tiles -> gin, AllGather -> gout, stream back
            the full (global-order) set of 32 tiles for the aggregations."""
            for t in range(LT):
                nc.sync.dma_start(
                    out=gin[t * P:(t + 1) * P, :], in_=own[t][:])
            nc.gpsimd.collective_compute(
                "AllGather", OP.bypass,
                replica_groups=[list(range(NCORES))],
                ins=[gin[:, :]], outs=[gout[:, :]])
            for t in range(NT):
                nc.sync.dma_start(out=full[t][:],
                                  in_=gout[t * P:(t + 1) * P, :])

        load_norm(xn_d, h_own, 0, 4, hfp)
        load_norm(xe_d, e_own, 0, 4, eearly)
        transpose_local(h_own, hTl)
        transpose_local(e_own, eTl)
        gather_x(h_own, gin_h, gout_h, h_sb)
        gather_x(e_own, gin_e, gout_e, e_sb)

        # module 1 needs no aggregate (only hTl/eTl) - start DVE early
        module(1, eTl, eTl, xatt_h, first=True)

        agg0_stack = ExitStack()
        agg0pool = agg0_stack.enter_context(tc.tile_pool(name="agg0p", bufs=1))
        agg0 = aggregate(0, h_sb, agg0pool)

        module(5, eTl, eTl, xatt_e, first=True)

        agg1 = aggregate(1, h_sb, agglate12)
        agg2 = aggregate(2, h_sb, agglate12)

        module(0, agg0, hTl, xatt_h, first=False)
        module(4, agg0, hTl, xatt_e, first=False)
        agg0_stack.close()
        hstack.close()

        agg3 = aggregate(3, e_sb, agglate34)
        agg4 = aggregate(4, e_sb, agglate34)

        module(2, eTl, agg1, xatt_h, first=False)
        module(3, hTl, agg3, xatt_h, first=False)
        eearly_stack.close()
        module(7, hTl, agg4, xatt_e, first=False)
        aggl34_stack.close()
        ffn(xatt_h, w1h_d, w2h_d, b1h_t, b2h_t, outh_d)
        module(6, eTl, agg2, xatt_e, first=False)
        aggl12_stack.close()
        ffn(xatt_e, w1e_d, w2e_d, b1e_t, b2e_t, oute_d)

    _split_big_waits(nc, mybir)
    return nc


def _get_program():
    if "nc" not in _PROGRAM_CACHE:
        _PROGRAM_CACHE["nc"] = _build_program()
    return _PROGRAM_CACHE["nc"]


def _prep_inputs(x_node, x_edge, adj, Wq, Wk, Wv,
                 proj_he_h, proj_eh_h, proj_he_e, proj_eh_e,
                 rms1_h, rms1_e, rms2_h,
                 w1_h, b1_h, w2_h, b2_h, w1_e, b1_e, w2_e, b2_e):
    """Per-core input dicts. Weight folding + row rotation happen here."""
    import ml_dtypes
    f = np.float32
    bf = ml_dtypes.bfloat16
    wsrc_q = [rms1_h, rms1_e, rms1_e, rms1_h, rms1_h, rms1_e, rms1_e, rms1_h]
    wsrc_k = [rms1_h, rms1_e, rms1_h, rms1_e, rms1_h, rms1_e, rms1_h, rms1_e]
    wqT = np.stack([(Wq[m].T * wsrc_q[m][:, None]) * 0.125 for m in range(H)])
    wkT = np.stack([Wk[m].T * wsrc_k[m][:, None] for m in range(H)])
    wvT = np.stack([Wv[m].T * rms1_h[:, None] for m in range(H)])
    w1hT = np.ascontiguousarray((w1_h * rms2_h[None, :]).T.astype(bf))
    w1eT = np.ascontiguousarray((w1_e * rms2_h[None, :]).T.astype(bf))
    w2hT = np.ascontiguousarray(w2_h.T.astype(bf))
    w2eT = np.ascontiguousarray(w2_e.T.astype(bf))
    mats = [adj, proj_eh_h, proj_eh_e, proj_he_h, proj_he_e]

    shared = dict(wqT=np.ascontiguousarray(wqT.astype(bf)),
                  wkT=np.ascontiguousarray(wkT.astype(bf)),
                  wvT=np.ascontiguousarray(wvT.astype(bf)),
                  w1hT=w1hT, w2hT=w2hT, w1eT=w1eT, w2eT=w2eT,
                  b1h=b1_h.astype(f), b2h=b2_h.astype(f),
                  b1e=b1_e.astype(f), b2e=b2_e.astype(f))
    in_maps = []
    for c in range(NCORES):
        r0 = c * RPC
        m = dict(shared)
        m["xn"] = np.ascontiguousarray(x_node[r0:r0 + RPC].astype(bf))
        m["xe"] = np.ascontiguousarray(x_edge[r0:r0 + RPC].astype(bf))
        for i, mat in enumerate(mats):
            # global (unrolled) contraction order; matches the AllGathered x
            m[f"mat{i}"] = np.ascontiguousarray(mat[r0:r0 + RPC].T.astype(bf))
        in_maps.append(m)
    return in_maps


def kernel(**inputs):
    from concourse.bass_utils import run_bass_kernel_spmd
    nc = _get_program()
    in_maps = _prep_inputs(**{k: np.asarray(v) for k, v in inputs.items()})
    res = run_bass_kernel_spmd(nc, in_maps, list(range(NCORES))).results
    x_h = np.concatenate([res[c]["outh"] for c in range(NCORES)], axis=0)
    x_e = np.concatenate([res[c]["oute"] for c in range(NCORES)], axis=0)
    return (x_h, x_e)



# revision 53
# speedup vs baseline: 1.1058x; 1.1058x over previous
"""Trainium2 Bass kernel for the gnn_message_passing block (8 NeuronCores).

Strategy (per core c, owning 512 global rows r = c*512..(c+1)*512):
  - Host rotates x_node/x_edge rows by -r0 so the owned rows sit first on
    every core (SPMD: one program, per-core data). All heavy DRAM streams
    (x, the five N x N mats, QKV/FFN weights) are pre-cast to bf16 on the
    host, halving HBM traffic.
  - Associativity: mat @ (x @ W.T) == (mat @ x) @ W.T, so the five big
    N x N aggregations (adj@h shared by modules 0/4, four proj@k inputs)
    are computed ONCE per core as row-blocks (mat[r].T streamed from DRAM
    as the bf16 moving operand; bf16 h/e tiles stationary).
  - rmsnorm weight vectors and the 1/sqrt(D) score scale are folded into
    the projection weights host-side; on-chip rmsnorm: Square+accum on
    ACT, rs = exp(-ln(mean+eps)/2) on ACT (stays in the natural_log_exp
    table set shared with the SDPA exp -> ~4 ACT table loads total), and
    the normalizing multiply on DVE (or ACT for bulk h tiles).
  - Per-node 8-head SDPA on DVE in fp16 with access patterns chosen to
    hit the 2x packed mode (microbenchmarked): products with contiguous
    dst + real-inner-dim srcs; halving trees with strided src but
    contiguous dst; exp/den/recip batched once per module (4 tiles) to
    avoid ACT table thrash; 1/den folded in after the AV sum.
  - q+k projections share one 2-bank PSUM tile and one ACT copy; k/v are
    stored d-major for the product patterns.
  - Emission order keeps each engine's in-order queue stall-free: bulk h
    norms run on ACT behind module 1's copies (agg0 unblocks while
    module-1 SDPA runs on DVE); e norms split around module 0; FFN-h sits
    between modules 7/6 so PE fills their SDPA windows; accumulation adds
    run on Pool (kept otherwise idle - Pool SBUF-port traffic degrades
    DVE 2x modes).
"""
import numpy as np

N = 4096
E = 512
H = 8
D = 64
FF = 2048
P = 128
NCORES = 8
RPC = N // NCORES  # 512 rows per core
NT = N // P        # 32 tiles over all nodes
LT = RPC // P      # 4 local tiles
EPS = float(np.finfo(np.float32).eps)

_PROGRAM_CACHE = {}


def _split_big_waits(nc, mybir):
    """walrus in this toolchain rejects multi-wait instructions; cap at 1
    (2 for EventSemaphore), chaining the excess as EventSemaphores."""
    for f in nc.m.functions:
        for bb in f.blocks:
            insts = list(bb.instructions)
            out = []
            changed = False
            for inst in insts:
                si = inst.sync_info
                waits = list(si.on_wait) if si and si.on_wait else []
                cap = 2 if isinstance(inst, mybir.InstEventSemaphore) else 1
                if len(waits) > cap:
                    extra, keep = waits[:-cap], waits[-cap:]
                    for ci in range(0, len(extra), 2):
                        ev = mybir.InstEventSemaphore(name=f"{inst.name}-evw{ci}")
                        ev.engine = inst.engine
                        ev.sync_info = mybir.SyncInfo(on_wait=extra[ci:ci + 2],
                                                      on_update=[])
                        out.append(ev)
                    si.on_wait = keep
                    changed = True
                out.append(inst)
            if changed:
                bb.instructions[:] = out


def _build_program():
    import concourse.bass as bass
    import concourse.tile as tile
    from concourse import mybir
    from concourse.masks import make_identity
    from contextlib import ExitStack

    f32 = mybir.dt.float32
    f32r = mybir.dt.float32r
    f16 = mybir.dt.float16
    bf16 = mybir.dt.bfloat16
    AF = mybir.ActivationFunctionType
    OP = mybir.AluOpType
    AX = mybir.AxisListType

    def bc(t, dims, off=0):
        return bass.AP(tensor=t.tensor, offset=t.offset + off,
                       ap=[list(t.ap[0])] + [[s, c] for (s, c) in dims])

    nc = bass.Bass()

    xn_d = nc.declare_dram_parameter("xn", [N, E], bf16, isOutput=False)
    xe_d = nc.declare_dram_parameter("xe", [N, E], bf16, isOutput=False)
    mat_d = [nc.declare_dram_parameter(f"mat{i}", [N, RPC], bf16, isOutput=False)
             for i in range(5)]
    wq_d = nc.declare_dram_parameter("wqT", [H, E, E], bf16, isOutput=False)
    wk_d = nc.declare_dram_parameter("wkT", [H, E, E], bf16, isOutput=False)
    wv_d = nc.declare_dram_parameter("wvT", [H, E, E], bf16, isOutput=False)
    w1h_d = nc.declare_dram_parameter("w1hT", [E, FF], bf16, isOutput=False)
    w2h_d = nc.declare_dram_parameter("w2hT", [FF, E], bf16, isOutput=False)
    w1e_d = nc.declare_dram_parameter("w1eT", [E, FF], bf16, isOutput=False)
    w2e_d = nc.declare_dram_parameter("w2eT", [FF, E], bf16, isOutput=False)
    b1h_d = nc.declare_dram_parameter("b1h", [FF], f32, isOutput=False)
    b2h_d = nc.declare_dram_parameter("b2h", [E], f32, isOutput=False)
    b1e_d = nc.declare_dram_parameter("b1e", [FF], f32, isOutput=False)
    b2e_d = nc.declare_dram_parameter("b2e", [E], f32, isOutput=False)
    outh_d = nc.declare_dram_parameter("outh", [RPC, E], f32, isOutput=True)
    oute_d = nc.declare_dram_parameter("oute", [RPC, E], f32, isOutput=True)

    with tile.TileContext(nc, pool_alloc_mode="queue") as tc, ExitStack() as ctx:
        consts = ctx.enter_context(tc.tile_pool(name="consts", bufs=1))
        ident = consts.tile([P, P], f32)
        make_identity(nc, ident)
        ident_bf = consts.tile([P, P], bf16)
        nc.scalar.copy(ident_bf[:], ident[:])
        ones1f = consts.tile([1, P], f32)
        nc.gpsimd.memset(ones1f, 1.0)
        ones1 = consts.tile([1, P], f32r)
        nc.scalar.copy(ones1[:], ones1f[:])
        eps_t = consts.tile([P, 1], f32)
        nc.vector.memset(eps_t, EPS)
        b1h_t = consts.tile([P, FF // P], f32)
        nc.gpsimd.dma_start(out=b1h_t, in_=b1h_d[:].rearrange("(c p) -> p c", p=P))
        b1e_t = consts.tile([P, FF // P], f32)
        nc.gpsimd.dma_start(out=b1e_t, in_=b1e_d[:].rearrange("(c p) -> p c", p=P))
        b2h_t = consts.tile([1, E], f32r)
        nc.gpsimd.dma_start(out=b2h_t, in_=b2h_d[:].rearrange("(a e) -> a e", a=1))
        b2e_t = consts.tile([1, E], f32r)
        nc.gpsimd.dma_start(out=b2e_t, in_=b2e_d[:].rearrange("(a e) -> a e", a=1))

        # whole-program pools
        locp = ctx.enter_context(tc.tile_pool(name="loc", bufs=1))
        attp = ctx.enter_context(tc.tile_pool(name="att", bufs=1))
        statp = ctx.enter_context(tc.tile_pool(name="stat", bufs=4))
        sqscp = ctx.enter_context(tc.tile_pool(name="sqsc", bufs=1))
        wpool = ctx.enter_context(tc.tile_pool(name="wts", bufs=1))
        qkvp = ctx.enter_context(tc.tile_pool(name="qkv", bufs=1))
        tmpp = ctx.enter_context(tc.tile_pool(name="sdtmp", bufs=1))
        smp = ctx.enter_context(tc.tile_pool(name="sdsm", bufs=2))
        psp = ctx.enter_context(tc.tile_pool(name="ps", bufs=1, space="PSUM"))

        hTl = [locp.tile([P, RPC], bf16, tag=f"hTl{fc}", name=f"hTl{fc}")
               for fc in range(4)]
        eTl = [locp.tile([P, RPC], bf16, tag=f"eTl{fc}", name=f"eTl{fc}")
               for fc in range(4)]
        xatt_h = [attp.tile([P, E], f32, tag=f"xh{t}", name=f"xh{t}")
                  for t in range(LT)]
        xatt_e = [attp.tile([P, E], f32, tag=f"xe{t}", name=f"xe{t}")
                  for t in range(LT)]

        def load_norm(x_dram, dst_tiles, t0, t1, xpool, on_act=False):
            """Stream x rows [t0*128, t1*128) in 4-tile DMAs; rmsnorm each.
            ACT computes sum-of-squares + Rsqrt; the normalizing multiply
            runs on DVE (fast path feeding transposes) or fully on ACT
            (bulk tiles, keeping the DVE queue free for SDPA)."""
            for g0 in range(t0, t1, 4):
                ng = min(4, t1 - g0)
                xg = xpool.tile([P, ng * E], bf16, tag="xing", name="xing")
                nc.sync.dma_start(
                    out=xg.rearrange("p (t e) -> p t e", e=E),
                    in_=x_dram[g0 * P:(g0 + ng) * P, :].rearrange(
                        "(t p) e -> p t e", p=P))
                ssq = statp.tile([P, 4], f32, tag="ssq", name="ssq")
                sc = sqscp.tile([P, E], f32, tag="sqsc", name="sqsc")
                for t in range(ng):
                    nc.scalar.activation(out=sc[:], in_=xg[:, t * E:(t + 1) * E],
                                         func=AF.Square,
                                         accum_out=ssq[:, t:t + 1])
                # rs = (mean+eps)^-1/2 = exp(-ln(mean+eps)/2): stays in the
                # natural_log_exp ACT table set shared with the SDPA exp
                lnv = statp.tile([P, 4], f32, tag="lnv", name="lnv")
                nc.scalar.activation(out=lnv[:, :ng], in_=ssq[:, :ng],
                                     func=AF.Ln, bias=eps_t[:], scale=1.0 / E)
                rs = statp.tile([P, 4], f32, tag="rs", name="rs")
                nc.scalar.activation(out=rs[:, :ng], in_=lnv[:, :ng],
                                     func=AF.Exp, scale=-0.5)
                for t in range(ng):
                    ti = g0 + t
                    if on_act:
                        nc.scalar.activation(out=dst_tiles[ti][:],
                                             in_=xg[:, t * E:(t + 1) * E],
                                             func=AF.Copy, scale=rs[:, t:t + 1])
                    else:
                        nc.vector.tensor_scalar_mul(
                            dst_tiles[ti][:], xg[:, t * E:(t + 1) * E],
                            rs[:, t:t + 1])

        def rmsnorm_tile(dst, src_ap):
            """dst = pure rmsnorm of node-major [128, 512] slice (FFN path)."""
            sc = sqscp.tile([P, E], f32, tag="sqsc", name="sqsc")
            ssq = statp.tile([P, 1], f32, tag="ssq2", name="ssq2")
            nc.scalar.activation(out=sc[:], in_=src_ap, func=AF.Square,
                                 accum_out=ssq[:])
            lnv = statp.tile([P, 1], f32, tag="lnv2", name="lnv2")
            nc.scalar.activation(out=lnv[:], in_=ssq[:], func=AF.Ln,
                                 bias=eps_t[:], scale=1.0 / E)
            rs = statp.tile([P, 1], f32, tag="rs2", name="rs2")
            nc.scalar.activation(out=rs[:], in_=lnv[:], func=AF.Exp, scale=-0.5)
            nc.vector.tensor_scalar_mul(dst[:], src_ap, rs[:])

        def transpose_local(srcs, dstT):
            for fc in range(4):
                ps = psp.tile([P, RPC], bf16, tag="projps", bufs=2, name="trps")
                for t in range(4):
                    nc.tensor.transpose(ps[:, t * P:(t + 1) * P],
                                        srcs[t][:, fc * P:(fc + 1) * P],
                                        ident_bf[:])
                nc.vector.tensor_copy(dstT[fc][:], ps[:])

        def aggregate(mi, lhs_tiles, aggpool):
            """returns 4 feature-major bf16 [128, 512] blocks of mat_mi @ x."""
            mst = ExitStack()
            matgp = mst.enter_context(tc.tile_pool(name=f"matg{mi}", bufs=2))
            pss = [psp.tile([P, E], f32, tag=f"agps{b}", name=f"agps{b}")
                   for b in range(4)]
            for g in range(8):
                mt = matgp.tile([P, 4 * RPC], bf16, tag="matg", name="matg")
                nc.sync.dma_start(
                    out=mt.rearrange("p (t e) -> p t e", e=RPC),
                    in_=mat_d[mi][g * 4 * P:(g + 1) * 4 * P, :].rearrange(
                        "(t p) e -> p t e", p=P))
                for t in range(4):
                    ti = g * 4 + t
                    for b in range(4):
                        nc.tensor.matmul(
                            pss[b][:],
                            lhsT=lhs_tiles[ti][:, b * P:(b + 1) * P],
                            rhs=mt[:, t * RPC:(t + 1) * RPC],
                            start=(ti == 0), stop=(ti == NT - 1))
            outt = []
            for b in range(4):
                at = aggpool.tile([P, E], bf16, tag=f"ag{mi}_{b}",
                                  name=f"ag{mi}_{b}")
                nc.scalar.copy(at[:], pss[b][:])
                outt.append(at)
            mst.close()
            return outt

        def module(m, qsrc, ksrc, branch_att, first):
            w_ts = {}
            for (dram, nm) in ((wq_d, "wq"), (wk_d, "wk"), (wv_d, "wv")):
                wt = wpool.tile([P, 4 * E], bf16, tag=nm, name=f"w_{nm}")
                nc.scalar.dma_start(
                    out=wt.rearrange("p (fc e) -> p fc e", e=E),
                    in_=dram[m].rearrange("(fc p) e -> p fc e", p=P))
                w_ts[nm] = wt

            # phase 1: per tile, project q/k/v and reduce scores into s_all
            s_all = smp.tile([P, LT * H * H], f16, tag="s", name="s")
            v_ts = []
            for t in range(LT):
                # q and k share one 2-bank psum tile and one ACT copy
                qk_ps = psp.tile([P, 2 * E], f32, tag="projqk", bufs=1,
                                 name="qkps")
                for (srcT, wnm, half) in ((qsrc, "wq", 0), (ksrc, "wk", 1)):
                    wt = w_ts[wnm]
                    for fc in range(4):
                        nc.tensor.matmul(
                            qk_ps[:, half * E:(half + 1) * E],
                            lhsT=srcT[fc][:, t * P:(t + 1) * P],
                            rhs=wt[:, fc * E:(fc + 1) * E],
                            start=(fc == 0), stop=(fc == 3))
                qk_t = qkvp.tile([P, 2 * E], f16, tag=f"qk_{t}", name=f"qk{t}")
                nc.scalar.copy(qk_t[:], qk_ps[:])
                q_t, k_t = qk_t, None  # k read via off=E on qk_t

                ps = psp.tile([P, E], f32, tag="projps", bufs=2, name="projps")
                wt = w_ts["wv"]
                for fc in range(4):
                    nc.tensor.matmul(
                        ps[:],
                        lhsT=hTl[fc][:, t * P:(t + 1) * P],
                        rhs=wt[:, fc * E:(fc + 1) * E],
                        start=(fc == 0), stop=(fc == 3))
                dt = qkvp.tile([P, E], f16, tag=f"wv_{t}", name=f"v{t}")
                # stored d-major, head-minor: dt[d*8+g] = ps[g*64+d]
                nc.scalar.copy(bc(dt, [(1, 8), (8, 64)]), ps[:])
                v_ts.append(dt)
                # scores, (h,g,d) layout: tmp[h*512+g*64+d] = q[h,d]*k[g,d]
                tmp = tmpp.tile([P, H * H * D], f16, tag="sdpa", bufs=2,
                                name="sdpa")
                nc.vector.tensor_tensor(
                    out=bc(tmp, [(512, 8), (64, 8), (1, 64)]),
                    in0=bc(q_t, [(64, 8), (0, 8), (1, 64)]),
                    in1=bc(q_t, [(0, 8), (64, 8), (1, 64)], off=E),
                    op=OP.mult)
                # halving tree over inner d: strided src, contiguous dst
                szs = (2048, 1024)
                bufs = [tmpp.tile([P, n], f16, tag=f"pp{n}", bufs=1,
                                  name=f"str{n}") for n in szs]
                src, run = tmp, 32
                for bi, n in enumerate(szs[:2]):
                    nc.vector.tensor_tensor(
                        out=bc(bufs[bi], [(1, n)]),
                        in0=bc(src, [(2 * run, 64), (1, run)]),
                        in1=bc(src, [(2 * run, 64), (1, run)], off=run),
                        op=OP.add)
                    src, run = bufs[bi], run // 2
                # remaining 16-wide groups in one 1x tensor_reduce
                with nc.allow_low_precision(reason="f16 score partial sums"):
                    nc.vector.tensor_reduce(
                        out=bc(s_all, [(1, 64)], off=t * H * H),
                        in_=bc(src, [(16, 64), (1, 16)]),
                        axis=AX.X, op=OP.add)

            # phase 2: one exp / den / recip for all 4 tiles
            ex_all = smp.tile([P, LT * H * H], f16, tag="ex", name="ex")
            nc.scalar.activation(out=ex_all[:], in_=s_all[:], func=AF.Exp)
            den = smp.tile([P, LT * H], f32, tag="den", name="den")
            nc.vector.tensor_reduce(
                out=den[:], in_=ex_all.rearrange("p (th g) -> p th g", g=H),
                axis=AX.X, op=OP.add)
            rden = smp.tile([P, LT * H], f16, tag="rden", name="rden")
            with nc.allow_low_precision(reason="f16 softmax denominators"):
                nc.vector.reciprocal(out=rden[:], in_=den[:])

            # phase 3: AV per tile with unnormalized ex; (h,d,g), g innermost
            for t in range(LT):
                v_t = v_ts[t]
                tmp2 = tmpp.tile([P, H * H * D], f16, tag="sdpa", bufs=2,
                                 name="sdpa2")
                nc.vector.tensor_tensor(
                    out=bc(tmp2, [(512, 8), (8, 64), (1, 8)]),
                    in0=bc(ex_all, [(8, 8), (0, 64), (1, 8)], off=t * H * H),
                    in1=bc(v_t, [(0, 8), (8, 64), (1, 8)]),
                    op=OP.mult)
                # pair tree over g: +4 (runs-4), +2 (runs-2), +1 (stride-2)
                av1 = tmpp.tile([P, 2048], f16, tag="pp2048", bufs=1, name="av1")
                nc.vector.tensor_tensor(
                    out=bc(av1, [(1, 2048)]),
                    in0=bc(tmp2, [(8, 512), (1, 4)]),
                    in1=bc(tmp2, [(8, 512), (1, 4)], off=4),
                    op=OP.add)
                av2 = tmpp.tile([P, 1024], f16, tag="pp1024", bufs=1, name="av2")
                nc.vector.tensor_tensor(
                    out=bc(av2, [(1, 1024)]),
                    in0=bc(av1, [(4, 512), (1, 2)]),
                    in1=bc(av1, [(4, 512), (1, 2)], off=2),
                    op=OP.add)
                avf = smp.tile([P, E], f16, tag="avf", name="avf")
                nc.vector.tensor_tensor(
                    out=avf[:],
                    in0=bc(av2, [(2, 512)]),
                    in1=bc(av2, [(2, 512)], off=1),
                    op=OP.add)
                # fold 1/den (per (n,h)) while writing the branch accumulator
                if first:
                    nc.vector.tensor_tensor(
                        out=branch_att[t][:], in0=avf[:],
                        in1=bc(rden, [(1, 8), (0, 64)], off=t * H),
                        op=OP.mult)
                else:
                    rt = smp.tile([P, E], f16, tag="avred", name="avred")
                    nc.vector.tensor_tensor(
                        out=rt[:], in0=avf[:],
                        in1=bc(rden, [(1, 8), (0, 64)], off=t * H),
                        op=OP.mult)
                    nc.gpsimd.tensor_tensor(out=branch_att[t][:],
                                            in0=branch_att[t][:], in1=rt[:],
                                            op=OP.add)

        def ffn(branch_att, w1_dram, w2_dram, b1_t, b2_t, out_dram):
            with tc.tile_pool(name="ffn_sb", bufs=1) as fsb, \
                 tc.tile_pool(name="ffn_xn", bufs=1) as fxn:
                xn_tiles = []
                for t in range(LT):
                    xt = fxn.tile([P, E], bf16, tag=f"fx{t}", name=f"fx{t}")
                    rmsnorm_tile(xt, branch_att[t][:])
                    xn_tiles.append(xt)
                xnT = []
                for fc in range(4):
                    ps = psp.tile([P, RPC], bf16, tag="agps0", name="ftr")
                    for t in range(4):
                        nc.tensor.transpose(ps[:, t * P:(t + 1) * P],
                                            xn_tiles[t][:, fc * P:(fc + 1) * P],
                                            ident_bf[:])
                    xt = fxn.tile([P, RPC], bf16, tag=f"fxT{fc}", name=f"fxT{fc}")
                    nc.scalar.copy(xt[:], ps[:])
                    xnT.append(xt)
                g1 = []
                HW1 = FF // 2
                for half in range(2):
                    w1_t = fsb.tile([P, 4 * HW1], bf16, tag="w1", name="w1")
                    nc.scalar.dma_start(
                        out=w1_t.rearrange("p (fc e) -> p fc e", e=HW1),
                        in_=w1_dram[:, half * HW1:(half + 1) * HW1].rearrange(
                            "(fc p) e -> p fc e", p=P))
                    for fb in range(HW1 // P):
                        ffb = half * (HW1 // P) + fb
                        ps = psp.tile([P, RPC], f32, tag=f"agps{1 + ffb % 2}",
                                      name="fps1")
                        for fc in range(4):
                            nc.tensor.matmul(
                                ps[:],
                                lhsT=w1_t[:, fc * HW1 + fb * P:
                                          fc * HW1 + (fb + 1) * P],
                                rhs=xnT[fc][:],
                                start=(fc == 0), stop=(fc == 3))
                        gt = fsb.tile([P, RPC], bf16, tag=f"g1_{ffb}",
                                      name=f"g1_{ffb}")
                        nc.scalar.activation(out=gt[:], in_=ps[:], func=AF.Gelu,
                                             bias=b1_t[:, ffb:ffb + 1], scale=1.0)
                        g1.append(gt)
                w2_t = fsb.tile([P, 16 * E], bf16, tag="w2", name="w2")
                nc.scalar.dma_start(
                    out=w2_t.rearrange("p (fc e) -> p fc e", e=E),
                    in_=w2_dram[:, :].rearrange("(fc p) e -> p fc e", p=P))
                ot = fsb.tile([P, 4 * E], f32, tag="fo", name="fo")
                for b in range(LT):
                    ps = psp.tile([P, E], f32, tag="agps3", name="fps2")
                    for ffc in range(FF // P):
                        nc.tensor.matmul(
                            ps[:],
                            lhsT=g1[ffc][:, b * P:(b + 1) * P],
                            rhs=w2_t[:, ffc * E:(ffc + 1) * E],
                            start=(ffc == 0), stop=False)
                    nc.tensor.matmul(ps[:], lhsT=ones1[:], rhs=b2_t[:],
                                     start=False, stop=True)
                    nc.scalar.copy(ot[:, b * E:(b + 1) * E], ps[:])
                nc.sync.dma_start(
                    out=out_dram[:, :].rearrange("(b p) e -> p b e", p=P),
                    in_=ot.rearrange("p (b e) -> p b e", e=E))

        # ======== emission order (the schedule) ========
        # Pool open/close must be LIFO: agglate (aggs 1-4) and eearly
        # outlive hfull; agg0/erest nest inside.
        aggl12_stack = ExitStack()
        agglate12 = aggl12_stack.enter_context(
            tc.tile_pool(name="agglate12", bufs=1))
        aggl34_stack = ExitStack()
        agglate34 = aggl34_stack.enter_context(
            tc.tile_pool(name="agglate34", bufs=1))
        eearly_stack = ExitStack()
        eearly = eearly_stack.enter_context(tc.tile_pool(name="eearly", bufs=1))
        hstack = ExitStack()
        hfp = hstack.enter_context(tc.tile_pool(name="hfull", bufs=1))

        h_sb = [hfp.tile([P, E], bf16, tag=f"h{t}", name=f"hsb{t}")
                for t in range(NT)]
        e_sb = [eearly.tile([P, E], bf16, tag=f"e{t}", name=f"esb{t}")
                for t in range(NT)]

        load_norm(xn_d, h_sb, 0, 4, hfp)
        load_norm(xe_d, e_sb, 0, 4, eearly)
        transpose_local(h_sb[:4], hTl)
        transpose_local(e_sb[:4], eTl)

        # module 1 needs no aggregate (only hTl/eTl) - start DVE early
        module(1, eTl, eTl, xatt_h, first=True)

        # bulk h norms run fully on ACT (after module 1's copies in the ACT
        # queue) so agg0's matmul groups unblock progressively while module
        # 1's SDPA runs on DVE
        load_norm(xn_d, h_sb, 4, NT, hfp, on_act=True)

        agg0_stack = ExitStack()
        agg0pool = agg0_stack.enter_context(tc.tile_pool(name="agg0p", bufs=1))
        agg0 = aggregate(0, h_sb, agg0pool)

        module(5, eTl, eTl, xatt_e, first=True)
        # e bulk split around module 0 so its ACT squares overlap module
        # SDPA windows instead of stalling the DVE norm-muls
        load_norm(xe_d, e_sb, 4, 16, eearly)

        agg1 = aggregate(1, h_sb, agglate12)
        agg2 = aggregate(2, h_sb, agglate12)

        module(0, agg0, hTl, xatt_h, first=False)
        load_norm(xe_d, e_sb, 16, NT, eearly)
        module(4, agg0, hTl, xatt_e, first=False)
        agg0_stack.close()
        hstack.close()

        agg3 = aggregate(3, e_sb, agglate34)
        agg4 = aggregate(4, e_sb, agglate34)

        module(2, eTl, agg1, xatt_h, first=False)
        module(3, hTl, agg3, xatt_h, first=False)
        eearly_stack.close()
        module(7, hTl, agg4, xatt_e, first=False)
        aggl34_stack.close()
        ffn(xatt_h, w1h_d, w2h_d, b1h_t, b2h_t, outh_d)
        module(6, eTl, agg2, xatt_e, first=False)
        aggl12_stack.close()
        ffn(xatt_e, w1e_d, w2e_d, b1e_t, b2e_t, oute_d)

    _split_big_waits(nc, mybir)
    return nc


def _get_program():
    if "nc" not in _PROGRAM_CACHE:
        _PROGRAM_CACHE["nc"] = _build_program()
    return _PROGRAM_CACHE["nc"]


def _prep_inputs(x_node, x_edge, adj, Wq, Wk, Wv,
                 proj_he_h, proj_eh_h, proj_he_e, proj_eh_e,
                 rms1_h, rms1_e, rms2_h,
                 w1_h, b1_h, w2_h, b2_h, w1_e, b1_e, w2_e, b2_e):
    """Per-core input dicts. Weight folding + row rotation happen here."""
    import ml_dtypes
    f = np.float32
    bf = ml_dtypes.bfloat16
    wsrc_q = [rms1_h, rms1_e, rms1_e, rms1_h, rms1_h, rms1_e, rms1_e, rms1_h]
    wsrc_k = [rms1_h, rms1_e, rms1_h, rms1_e, rms1_h, rms1_e, rms1_h, rms1_e]
    wqT = np.stack([(Wq[m].T * wsrc_q[m][:, None]) * 0.125 for m in range(H)])
    wkT = np.stack([Wk[m].T * wsrc_k[m][:, None] for m in range(H)])
    wvT = np.stack([Wv[m].T * rms1_h[:, None] for m in range(H)])
    w1hT = np.ascontiguousarray((w1_h * rms2_h[None, :]).T.astype(bf))
    w1eT = np.ascontiguousarray((w1_e * rms2_h[None, :]).T.astype(bf))
    w2hT = np.ascontiguousarray(w2_h.T.astype(bf))
    w2eT = np.ascontiguousarray(w2_e.T.astype(bf))
    mats = [adj, proj_eh_h, proj_eh_e, proj_he_h, proj_he_e]

    shared = dict(wqT=np.ascontiguousarray(wqT.astype(bf)),
                  wkT=np.ascontiguousarray(wkT.astype(bf)),
                  wvT=np.ascontiguousarray(wvT.astype(bf)),
                  w1hT=w1hT, w2hT=w2hT, w1eT=w1eT, w2eT=w2eT,
                  b1h=b1_h.astype(f), b2h=b2_h.astype(f),
                  b1e=b1_e.astype(f), b2e=b2_e.astype(f))
    in_maps = []
    for c in range(NCORES):
        r0 = c * RPC
        m = dict(shared)
        m["xn"] = np.ascontiguousarray(np.roll(x_node, -r0, axis=0).astype(bf))
        m["xe"] = np.ascontiguousarray(np.roll(x_edge, -r0, axis=0).astype(bf))
        for i, mat in enumerate(mats):
            mt = mat[r0:r0 + RPC].T.astype(bf)  # [N, RPC]
            m[f"mat{i}"] = np.ascontiguousarray(np.roll(mt, -r0, axis=0))
        in_maps.append(m)
    return in_maps


def kernel(**inputs):
    from concourse.bass_utils import run_bass_kernel_spmd
    nc = _get_program()
    in_maps = _prep_inputs(**{k: np.asarray(v) for k, v in inputs.items()})
    res = run_bass_kernel_spmd(nc, in_maps, list(range(NCORES))).results
    x_h = np.concatenate([res[c]["outh"] for c in range(NCORES)], axis=0)
    x_e = np.concatenate([res[c]["oute"] for c in range(NCORES)], axis=0)
    return (x_h, x_e)



# revision 59
# speedup vs baseline: 1.1399x; 1.0308x over previous
"""Trainium2 Bass kernel for the gnn_message_passing block (8 NeuronCores).

Strategy (per core c, owning 512 global rows r = c*512..(c+1)*512):
  - Host rotates x_node/x_edge rows by -r0 so the owned rows sit first on
    every core (SPMD: one program, per-core data). All heavy DRAM streams
    (x, the five N x N mats, QKV/FFN weights) are pre-cast to bf16 on the
    host, halving HBM traffic.
  - Associativity: mat @ (x @ W.T) == (mat @ x) @ W.T, so the five big
    N x N aggregations (adj@h shared by modules 0/4, four proj@k inputs)
    are computed ONCE per core as row-blocks (mat[r].T streamed from DRAM
    as the bf16 moving operand; bf16 h/e tiles stationary).
  - rmsnorm weight vectors and the 1/sqrt(D) score scale are folded into
    the projection weights host-side; on-chip rmsnorm: Square+accum on
    ACT, rs = exp(-ln(mean+eps)/2) on ACT (stays in the natural_log_exp
    table set shared with the SDPA exp -> ~4 ACT table loads total), and
    the normalizing multiply on DVE (or ACT for bulk h tiles).
  - Per-node 8-head SDPA on DVE in fp16 with access patterns chosen to
    hit the 2x packed mode (microbenchmarked): products with contiguous
    dst + real-inner-dim srcs; halving trees with strided src but
    contiguous dst; exp/den/recip batched once per module (4 tiles) to
    avoid ACT table thrash; 1/den folded in after the AV sum.
  - q+k projections share one 2-bank PSUM tile and one ACT copy; k/v are
    stored d-major for the product patterns.
  - Emission order keeps each engine's in-order queue stall-free: bulk h
    norms run on ACT behind module 1's copies (agg0 unblocks while
    module-1 SDPA runs on DVE); e norms split around module 0; FFN-h sits
    between modules 7/6 so PE fills their SDPA windows; accumulation adds
    run on Pool (kept otherwise idle - Pool SBUF-port traffic degrades
    DVE 2x modes).
"""
import numpy as np

N = 4096
E = 512
H = 8
D = 64
FF = 2048
P = 128
NCORES = 8
RPC = N // NCORES  # 512 rows per core
NT = N // P        # 32 tiles over all nodes
LT = RPC // P      # 4 local tiles
EPS = float(np.finfo(np.float32).eps)

_PROGRAM_CACHE = {}


def _split_big_waits(nc, mybir):
    """walrus in this toolchain rejects multi-wait instructions; cap at 1
    (2 for EventSemaphore), chaining the excess as EventSemaphores."""
    for f in nc.m.functions:
        for bb in f.blocks:
            insts = list(bb.instructions)
            out = []
            changed = False
            for inst in insts:
                si = inst.sync_info
                waits = list(si.on_wait) if si and si.on_wait else []
                cap = 2 if isinstance(inst, mybir.InstEventSemaphore) else 1
                if len(waits) > cap:
                    extra, keep = waits[:-cap], waits[-cap:]
                    for ci in range(0, len(extra), 2):
                        ev = mybir.InstEventSemaphore(name=f"{inst.name}-evw{ci}")
                        ev.engine = inst.engine
                        ev.sync_info = mybir.SyncInfo(on_wait=extra[ci:ci + 2],
                                                      on_update=[])
                        out.append(ev)
                    si.on_wait = keep
                    changed = True
                out.append(inst)
            if changed:
                bb.instructions[:] = out


def _build_program():
    import concourse.bass as bass
    import concourse.tile as tile
    from concourse import mybir
    from concourse.masks import make_identity
    from contextlib import ExitStack

    f32 = mybir.dt.float32
    f32r = mybir.dt.float32r
    f16 = mybir.dt.float16
    bf16 = mybir.dt.bfloat16
    AF = mybir.ActivationFunctionType
    OP = mybir.AluOpType
    AX = mybir.AxisListType

    def bc(t, dims, off=0):
        return bass.AP(tensor=t.tensor, offset=t.offset + off,
                       ap=[list(t.ap[0])] + [[s, c] for (s, c) in dims])

    nc = bass.Bass()

    xn_d = nc.declare_dram_parameter("xn", [N, E], bf16, isOutput=False)
    xe_d = nc.declare_dram_parameter("xe", [N, E], bf16, isOutput=False)
    mat_d = [nc.declare_dram_parameter(f"mat{i}", [N, RPC], bf16, isOutput=False)
             for i in range(5)]
    wq_d = nc.declare_dram_parameter("wqT", [H, E, E], bf16, isOutput=False)
    wk_d = nc.declare_dram_parameter("wkT", [H, E, E], bf16, isOutput=False)
    wv_d = nc.declare_dram_parameter("wvT", [H, E, E], bf16, isOutput=False)
    w1h_d = nc.declare_dram_parameter("w1hT", [E, FF], bf16, isOutput=False)
    w2h_d = nc.declare_dram_parameter("w2hT", [FF, E], bf16, isOutput=False)
    w1e_d = nc.declare_dram_parameter("w1eT", [E, FF], bf16, isOutput=False)
    w2e_d = nc.declare_dram_parameter("w2eT", [FF, E], bf16, isOutput=False)
    b1h_d = nc.declare_dram_parameter("b1h", [FF], f32, isOutput=False)
    b2h_d = nc.declare_dram_parameter("b2h", [E], f32, isOutput=False)
    b1e_d = nc.declare_dram_parameter("b1e", [FF], f32, isOutput=False)
    b2e_d = nc.declare_dram_parameter("b2e", [E], f32, isOutput=False)
    outh_d = nc.declare_dram_parameter("outh", [RPC, E], f32, isOutput=True)
    oute_d = nc.declare_dram_parameter("oute", [RPC, E], f32, isOutput=True)

    with tile.TileContext(nc, pool_alloc_mode="queue") as tc, ExitStack() as ctx:
        consts = ctx.enter_context(tc.tile_pool(name="consts", bufs=1))
        ident = consts.tile([P, P], f32)
        make_identity(nc, ident)
        ident_bf = consts.tile([P, P], bf16)
        nc.scalar.copy(ident_bf[:], ident[:])
        ones1f = consts.tile([1, P], f32)
        nc.gpsimd.memset(ones1f, 1.0)
        ones1 = consts.tile([1, P], f32r)
        nc.scalar.copy(ones1[:], ones1f[:])
        eps_t = consts.tile([P, 1], f32)
        nc.vector.memset(eps_t, EPS)
        b1h_t = consts.tile([P, FF // P], f32)
        nc.gpsimd.dma_start(out=b1h_t, in_=b1h_d[:].rearrange("(c p) -> p c", p=P))
        b1e_t = consts.tile([P, FF // P], f32)
        nc.gpsimd.dma_start(out=b1e_t, in_=b1e_d[:].rearrange("(c p) -> p c", p=P))
        b2h_t = consts.tile([1, E], f32r)
        nc.gpsimd.dma_start(out=b2h_t, in_=b2h_d[:].rearrange("(a e) -> a e", a=1))
        b2e_t = consts.tile([1, E], f32r)
        nc.gpsimd.dma_start(out=b2e_t, in_=b2e_d[:].rearrange("(a e) -> a e", a=1))

        # whole-program pools
        locp = ctx.enter_context(tc.tile_pool(name="loc", bufs=1))
        attp = ctx.enter_context(tc.tile_pool(name="att", bufs=1))
        statp = ctx.enter_context(tc.tile_pool(name="stat", bufs=4))
        sqscp = ctx.enter_context(tc.tile_pool(name="sqsc", bufs=1))
        wpool = ctx.enter_context(tc.tile_pool(name="wts", bufs=1))
        qkvp = ctx.enter_context(tc.tile_pool(name="qkv", bufs=1))
        tmpp = ctx.enter_context(tc.tile_pool(name="sdtmp", bufs=1))
        smp = ctx.enter_context(tc.tile_pool(name="sdsm", bufs=2))
        psp = ctx.enter_context(tc.tile_pool(name="ps", bufs=1, space="PSUM"))

        hTl = [locp.tile([P, RPC], bf16, tag=f"hTl{fc}", name=f"hTl{fc}")
               for fc in range(4)]
        eTl = [locp.tile([P, RPC], bf16, tag=f"eTl{fc}", name=f"eTl{fc}")
               for fc in range(4)]
        xatt_h = [attp.tile([P, E], f32, tag=f"xh{t}", name=f"xh{t}")
                  for t in range(LT)]
        xatt_e = [attp.tile([P, E], f32, tag=f"xe{t}", name=f"xe{t}")
                  for t in range(LT)]

        def load_norm(x_dram, dst_tiles, t0, t1, xpool, on_act=False,
                      on_dve_stt=False):
            """Stream x rows [t0*128, t1*128) in 4-tile DMAs; rmsnorm each.
            ACT computes sum-of-squares + Rsqrt; the normalizing multiply
            runs on DVE (fast path feeding transposes) or fully on ACT
            (bulk tiles, keeping the DVE queue free for SDPA)."""
            for g0 in range(t0, t1, 4):
                ng = min(4, t1 - g0)
                xg = xpool.tile([P, ng * E], bf16, tag="xing", name="xing")
                nc.sync.dma_start(
                    out=xg.rearrange("p (t e) -> p t e", e=E),
                    in_=x_dram[g0 * P:(g0 + ng) * P, :].rearrange(
                        "(t p) e -> p t e", p=P))
                ssq = statp.tile([P, 4], f32, tag="ssq", name="ssq")
                if on_dve_stt:
                    # sum-of-squares on DVE: decoupled from ACT's copy queue
                    sc = sqscp.tile([P, E], bf16, tag="sqscd", name="sqscd")
                    for t in range(ng):
                        nc.vector.scalar_tensor_tensor(
                            out=sc[:], in0=xg[:, t * E:(t + 1) * E], scalar=1.0,
                            in1=xg[:, t * E:(t + 1) * E],
                            op0=OP.mult, op1=OP.mult,
                            accum_out=ssq[:, t:t + 1])
                else:
                    sc = sqscp.tile([P, E], f32, tag="sqsc", name="sqsc")
                    for t in range(ng):
                        nc.scalar.activation(out=sc[:],
                                             in_=xg[:, t * E:(t + 1) * E],
                                             func=AF.Square,
                                             accum_out=ssq[:, t:t + 1])
                # rs = (mean+eps)^-1/2 = exp(-ln(mean+eps)/2): stays in the
                # natural_log_exp ACT table set shared with the SDPA exp
                lnv = statp.tile([P, 4], f32, tag="lnv", name="lnv")
                nc.scalar.activation(out=lnv[:, :ng], in_=ssq[:, :ng],
                                     func=AF.Ln, bias=eps_t[:], scale=1.0 / E)
                rs = statp.tile([P, 4], f32, tag="rs", name="rs")
                nc.scalar.activation(out=rs[:, :ng], in_=lnv[:, :ng],
                                     func=AF.Exp, scale=-0.5)
                for t in range(ng):
                    ti = g0 + t
                    if on_act:
                        nc.scalar.activation(out=dst_tiles[ti][:],
                                             in_=xg[:, t * E:(t + 1) * E],
                                             func=AF.Copy, scale=rs[:, t:t + 1])
                    else:
                        nc.vector.tensor_scalar_mul(
                            dst_tiles[ti][:], xg[:, t * E:(t + 1) * E],
                            rs[:, t:t + 1])

        def rmsnorm_tile(dst, src_ap):
            """dst = pure rmsnorm of node-major [128, 512] slice (FFN path)."""
            sc = sqscp.tile([P, E], f32, tag="sqsc", name="sqsc")
            ssq = statp.tile([P, 1], f32, tag="ssq2", name="ssq2")
            nc.scalar.activation(out=sc[:], in_=src_ap, func=AF.Square,
                                 accum_out=ssq[:])
            lnv = statp.tile([P, 1], f32, tag="lnv2", name="lnv2")
            nc.scalar.activation(out=lnv[:], in_=ssq[:], func=AF.Ln,
                                 bias=eps_t[:], scale=1.0 / E)
            rs = statp.tile([P, 1], f32, tag="rs2", name="rs2")
            nc.scalar.activation(out=rs[:], in_=lnv[:], func=AF.Exp, scale=-0.5)
            nc.vector.tensor_scalar_mul(dst[:], src_ap, rs[:])

        def transpose_local(srcs, dstT):
            for fc in range(4):
                ps = psp.tile([P, RPC], bf16, tag="projps", bufs=2, name="trps")
                for t in range(4):
                    nc.tensor.transpose(ps[:, t * P:(t + 1) * P],
                                        srcs[t][:, fc * P:(fc + 1) * P],
                                        ident_bf[:])
                nc.vector.tensor_copy(dstT[fc][:], ps[:])

        def aggregate(mi, lhs_tiles, aggpool):
            """returns 4 feature-major bf16 [128, 512] blocks of mat_mi @ x."""
            mst = ExitStack()
            matgp = mst.enter_context(tc.tile_pool(name=f"matg{mi}", bufs=2))
            pss = [psp.tile([P, E], f32, tag=f"agps{b}", name=f"agps{b}")
                   for b in range(4)]
            for g in range(8):
                mt = matgp.tile([P, 4 * RPC], bf16, tag="matg", name="matg")
                nc.sync.dma_start(
                    out=mt.rearrange("p (t e) -> p t e", e=RPC),
                    in_=mat_d[mi][g * 4 * P:(g + 1) * 4 * P, :].rearrange(
                        "(t p) e -> p t e", p=P))
                for t in range(4):
                    ti = g * 4 + t
                    for b in range(4):
                        nc.tensor.matmul(
                            pss[b][:],
                            lhsT=lhs_tiles[ti][:, b * P:(b + 1) * P],
                            rhs=mt[:, t * RPC:(t + 1) * RPC],
                            start=(ti == 0), stop=(ti == NT - 1))
            outt = []
            for b in range(4):
                at = aggpool.tile([P, E], bf16, tag=f"ag{mi}_{b}",
                                  name=f"ag{mi}_{b}")
                nc.scalar.copy(at[:], pss[b][:])
                outt.append(at)
            mst.close()
            return outt

        def module(m, qsrc, ksrc, branch_att, first):
            w_ts = {}
            for (dram, nm) in ((wq_d, "wq"), (wk_d, "wk"), (wv_d, "wv")):
                wt = wpool.tile([P, 4 * E], bf16, tag=nm, name=f"w_{nm}")
                nc.scalar.dma_start(
                    out=wt.rearrange("p (fc e) -> p fc e", e=E),
                    in_=dram[m].rearrange("(fc p) e -> p fc e", p=P))
                w_ts[nm] = wt

            # phase 1: per tile, project q/k/v and reduce scores into s_all
            s_all = smp.tile([P, LT * H * H], f16, tag="s", name="s")
            v_ts = []
            for t in range(LT):
                # q and k share one 2-bank psum tile and one ACT copy
                qk_ps = psp.tile([P, 2 * E], f32, tag="projqk", bufs=1,
                                 name="qkps")
                for (srcT, wnm, half) in ((qsrc, "wq", 0), (ksrc, "wk", 1)):
                    wt = w_ts[wnm]
                    for fc in range(4):
                        nc.tensor.matmul(
                            qk_ps[:, half * E:(half + 1) * E],
                            lhsT=srcT[fc][:, t * P:(t + 1) * P],
                            rhs=wt[:, fc * E:(fc + 1) * E],
                            start=(fc == 0), stop=(fc == 3))
                qk_t = qkvp.tile([P, 2 * E], f16, tag=f"qk_{t}", name=f"qk{t}")
                nc.scalar.copy(qk_t[:], qk_ps[:])
                q_t, k_t = qk_t, None  # k read via off=E on qk_t

                ps = psp.tile([P, E], f32, tag="projps", bufs=2, name="projps")
                wt = w_ts["wv"]
                for fc in range(4):
                    nc.tensor.matmul(
                        ps[:],
                        lhsT=hTl[fc][:, t * P:(t + 1) * P],
                        rhs=wt[:, fc * E:(fc + 1) * E],
                        start=(fc == 0), stop=(fc == 3))
                dt = qkvp.tile([P, E], f16, tag=f"wv_{t}", name=f"v{t}")
                # stored d-major, head-minor: dt[d*8+g] = ps[g*64+d]
                nc.scalar.copy(bc(dt, [(1, 8), (8, 64)]), ps[:])
                v_ts.append(dt)
                # scores, (h,g,d) layout: tmp[h*512+g*64+d] = q[h,d]*k[g,d]
                tmp = tmpp.tile([P, H * H * D], f16, tag="sdpa", bufs=2,
                                name="sdpa")
                nc.vector.tensor_tensor(
                    out=bc(tmp, [(512, 8), (64, 8), (1, 64)]),
                    in0=bc(q_t, [(64, 8), (0, 8), (1, 64)]),
                    in1=bc(q_t, [(0, 8), (64, 8), (1, 64)], off=E),
                    op=OP.mult)
                # halving tree over inner d: strided src, contiguous dst
                szs = (2048, 1024)
                bufs = [tmpp.tile([P, n], f16, tag=f"pp{n}", bufs=1,
                                  name=f"str{n}") for n in szs]
                src, run = tmp, 32
                for bi, n in enumerate(szs[:2]):
                    nc.vector.tensor_tensor(
                        out=bc(bufs[bi], [(1, n)]),
                        in0=bc(src, [(2 * run, 64), (1, run)]),
                        in1=bc(src, [(2 * run, 64), (1, run)], off=run),
                        op=OP.add)
                    src, run = bufs[bi], run // 2
                # remaining 16-wide groups in one 1x tensor_reduce
                with nc.allow_low_precision(reason="f16 score partial sums"):
                    nc.vector.tensor_reduce(
                        out=bc(s_all, [(1, 64)], off=t * H * H),
                        in_=bc(src, [(16, 64), (1, 16)]),
                        axis=AX.X, op=OP.add)

            # phase 2: one exp / den / recip for all 4 tiles
            ex_all = smp.tile([P, LT * H * H], f16, tag="ex", name="ex")
            nc.scalar.activation(out=ex_all[:], in_=s_all[:], func=AF.Exp)
            den = smp.tile([P, LT * H], f32, tag="den", name="den")
            nc.vector.tensor_reduce(
                out=den[:], in_=ex_all.rearrange("p (th g) -> p th g", g=H),
                axis=AX.X, op=OP.add)
            rden = smp.tile([P, LT * H], f16, tag="rden", name="rden")
            with nc.allow_low_precision(reason="f16 softmax denominators"):
                nc.vector.reciprocal(out=rden[:], in_=den[:])
            # normalize ex once for all 4 tiles: a[t,h,g] = ex[t,h,g]/den[t,h]
            a_all = smp.tile([P, LT * H * H], f16, tag="aall", name="aall")
            nc.vector.tensor_tensor(
                out=a_all[:], in0=ex_all[:],
                in1=bc(rden, [(8, LT), (1, 8), (0, 8)]),
                op=OP.mult)

            # phase 3: AV per tile with unnormalized ex; (h,d,g), g innermost
            for t in range(LT):
                v_t = v_ts[t]
                tmp2 = tmpp.tile([P, H * H * D], f16, tag="sdpa", bufs=2,
                                 name="sdpa2")
                nc.vector.tensor_tensor(
                    out=bc(tmp2, [(512, 8), (8, 64), (1, 8)]),
                    in0=bc(a_all, [(8, 8), (0, 64), (1, 8)], off=t * H * H),
                    in1=bc(v_t, [(0, 8), (8, 64), (1, 8)]),
                    op=OP.mult)
                # pair tree over g: +4 (runs-4), +2 (runs-2), +1 (stride-2)
                av1 = tmpp.tile([P, 2048], f16, tag="pp2048", bufs=1, name="av1")
                nc.vector.tensor_tensor(
                    out=bc(av1, [(1, 2048)]),
                    in0=bc(tmp2, [(8, 512), (1, 4)]),
                    in1=bc(tmp2, [(8, 512), (1, 4)], off=4),
                    op=OP.add)
                av2 = tmpp.tile([P, 1024], f16, tag="pp1024", bufs=1, name="av2")
                nc.vector.tensor_tensor(
                    out=bc(av2, [(1, 1024)]),
                    in0=bc(av1, [(4, 512), (1, 2)]),
                    in1=bc(av1, [(4, 512), (1, 2)], off=2),
                    op=OP.add)
                # last pair-sum writes the branch accumulator directly
                if first:
                    nc.vector.tensor_tensor(
                        out=branch_att[t][:],
                        in0=bc(av2, [(2, 512)]),
                        in1=bc(av2, [(2, 512)], off=1),
                        op=OP.add)
                else:
                    rt = smp.tile([P, E], f16, tag="avred", name="avred")
                    nc.vector.tensor_tensor(
                        out=rt[:],
                        in0=bc(av2, [(2, 512)]),
                        in1=bc(av2, [(2, 512)], off=1),
                        op=OP.add)
                    nc.gpsimd.tensor_tensor(out=branch_att[t][:],
                                            in0=branch_att[t][:], in1=rt[:],
                                            op=OP.add)

        def ffn(branch_att, w1_dram, w2_dram, b1_t, b2_t, out_dram):
            with tc.tile_pool(name="ffn_sb", bufs=1) as fsb, \
                 tc.tile_pool(name="ffn_xn", bufs=1) as fxn:
                xn_tiles = []
                for t in range(LT):
                    xt = fxn.tile([P, E], bf16, tag=f"fx{t}", name=f"fx{t}")
                    rmsnorm_tile(xt, branch_att[t][:])
                    xn_tiles.append(xt)
                xnT = []
                for fc in range(4):
                    ps = psp.tile([P, RPC], bf16, tag="agps0", name="ftr")
                    for t in range(4):
                        nc.tensor.transpose(ps[:, t * P:(t + 1) * P],
                                            xn_tiles[t][:, fc * P:(fc + 1) * P],
                                            ident_bf[:])
                    xt = fxn.tile([P, RPC], bf16, tag=f"fxT{fc}", name=f"fxT{fc}")
                    nc.scalar.copy(xt[:], ps[:])
                    xnT.append(xt)
                g1 = []
                HW1 = FF // 2
                for half in range(2):
                    w1_t = fsb.tile([P, 4 * HW1], bf16, tag="w1", name="w1")
                    nc.scalar.dma_start(
                        out=w1_t.rearrange("p (fc e) -> p fc e", e=HW1),
                        in_=w1_dram[:, half * HW1:(half + 1) * HW1].rearrange(
                            "(fc p) e -> p fc e", p=P))
                    for fb in range(HW1 // P):
                        ffb = half * (HW1 // P) + fb
                        ps = psp.tile([P, RPC], f32, tag=f"agps{1 + ffb % 2}",
                                      name="fps1")
                        for fc in range(4):
                            nc.tensor.matmul(
                                ps[:],
                                lhsT=w1_t[:, fc * HW1 + fb * P:
                                          fc * HW1 + (fb + 1) * P],
                                rhs=xnT[fc][:],
                                start=(fc == 0), stop=(fc == 3))
                        gt = fsb.tile([P, RPC], bf16, tag=f"g1_{ffb}",
                                      name=f"g1_{ffb}")
                        nc.scalar.activation(out=gt[:], in_=ps[:], func=AF.Gelu,
                                             bias=b1_t[:, ffb:ffb + 1], scale=1.0)
                        g1.append(gt)
                w2_t = fsb.tile([P, 16 * E], bf16, tag="w2", name="w2")
                nc.scalar.dma_start(
                    out=w2_t.rearrange("p (fc e) -> p fc e", e=E),
                    in_=w2_dram[:, :].rearrange("(fc p) e -> p fc e", p=P))
                ot = fsb.tile([P, 4 * E], f32, tag="fo", name="fo")
                for b in range(LT):
                    ps = psp.tile([P, E], f32, tag="agps3", name="fps2")
                    for ffc in range(FF // P):
                        nc.tensor.matmul(
                            ps[:],
                            lhsT=g1[ffc][:, b * P:(b + 1) * P],
                            rhs=w2_t[:, ffc * E:(ffc + 1) * E],
                            start=(ffc == 0), stop=False)
                    nc.tensor.matmul(ps[:], lhsT=ones1[:], rhs=b2_t[:],
                                     start=False, stop=True)
                    nc.scalar.copy(ot[:, b * E:(b + 1) * E], ps[:])
                nc.sync.dma_start(
                    out=out_dram[:, :].rearrange("(b p) e -> p b e", p=P),
                    in_=ot.rearrange("p (b e) -> p b e", e=E))

        # ======== emission order (the schedule) ========
        # Pool open/close must be LIFO: agglate (aggs 1-4) and eearly
        # outlive hfull; agg0/erest nest inside.
        aggl12_stack = ExitStack()
        agglate12 = aggl12_stack.enter_context(
            tc.tile_pool(name="agglate12", bufs=1))
        aggl34_stack = ExitStack()
        agglate34 = aggl34_stack.enter_context(
            tc.tile_pool(name="agglate34", bufs=1))
        eearly_stack = ExitStack()
        eearly = eearly_stack.enter_context(tc.tile_pool(name="eearly", bufs=1))
        hstack = ExitStack()
        hfp = hstack.enter_context(tc.tile_pool(name="hfull", bufs=1))

        h_sb = [hfp.tile([P, E], bf16, tag=f"h{t}", name=f"hsb{t}")
                for t in range(NT)]
        e_sb = [eearly.tile([P, E], bf16, tag=f"e{t}", name=f"esb{t}")
                for t in range(NT)]

        load_norm(xn_d, h_sb, 0, 4, hfp)
        load_norm(xe_d, e_sb, 0, 4, eearly)
        transpose_local(h_sb[:4], hTl)
        transpose_local(e_sb[:4], eTl)

        # module 1 needs no aggregate (only hTl/eTl) - start DVE early
        module(1, eTl, eTl, xatt_h, first=True)

        # bulk h norms run fully on ACT (after module 1's copies in the ACT
        # queue) so agg0's matmul groups unblock progressively while module
        # 1's SDPA runs on DVE
        load_norm(xn_d, h_sb, 4, NT, hfp, on_act=True)

        agg0_stack = ExitStack()
        agg0pool = agg0_stack.enter_context(tc.tile_pool(name="agg0p", bufs=1))
        agg0 = aggregate(0, h_sb, agg0pool)

        module(5, eTl, eTl, xatt_e, first=True)
        # e bulk with DVE sum-of-squares so the norm chain doesn't queue
        # behind copies on ACT; split around module 0
        load_norm(xe_d, e_sb, 4, 16, eearly, on_dve_stt=True)

        agg1 = aggregate(1, h_sb, agglate12)
        agg2 = aggregate(2, h_sb, agglate12)

        module(0, agg0, hTl, xatt_h, first=False)
        load_norm(xe_d, e_sb, 16, NT, eearly, on_dve_stt=True)
        module(4, agg0, hTl, xatt_e, first=False)
        agg0_stack.close()
        hstack.close()

        agg3 = aggregate(3, e_sb, agglate34)
        agg4 = aggregate(4, e_sb, agglate34)

        module(2, eTl, agg1, xatt_h, first=False)
        module(3, hTl, agg3, xatt_h, first=False)
        eearly_stack.close()
        module(7, hTl, agg4, xatt_e, first=False)
        aggl34_stack.close()
        ffn(xatt_h, w1h_d, w2h_d, b1h_t, b2h_t, outh_d)
        module(6, eTl, agg2, xatt_e, first=False)
        aggl12_stack.close()
        ffn(xatt_e, w1e_d, w2e_d, b1e_t, b2e_t, oute_d)

    _split_big_waits(nc, mybir)
    return nc


def _get_program():
    if "nc" not in _PROGRAM_CACHE:
        _PROGRAM_CACHE["nc"] = _build_program()
    return _PROGRAM_CACHE["nc"]


def _prep_inputs(x_node, x_edge, adj, Wq, Wk, Wv,
                 proj_he_h, proj_eh_h, proj_he_e, proj_eh_e,
                 rms1_h, rms1_e, rms2_h,
                 w1_h, b1_h, w2_h, b2_h, w1_e, b1_e, w2_e, b2_e):
    """Per-core input dicts. Weight folding + row rotation happen here."""
    import ml_dtypes
    f = np.float32
    bf = ml_dtypes.bfloat16
    wsrc_q = [rms1_h, rms1_e, rms1_e, rms1_h, rms1_h, rms1_e, rms1_e, rms1_h]
    wsrc_k = [rms1_h, rms1_e, rms1_h, rms1_e, rms1_h, rms1_e, rms1_h, rms1_e]
    wqT = np.stack([(Wq[m].T * wsrc_q[m][:, None]) * 0.125 for m in range(H)])
    wkT = np.stack([Wk[m].T * wsrc_k[m][:, None] for m in range(H)])
    wvT = np.stack([Wv[m].T * rms1_h[:, None] for m in range(H)])
    w1hT = np.ascontiguousarray((w1_h * rms2_h[None, :]).T.astype(bf))
    w1eT = np.ascontiguousarray((w1_e * rms2_h[None, :]).T.astype(bf))
    w2hT = np.ascontiguousarray(w2_h.T.astype(bf))
    w2eT = np.ascontiguousarray(w2_e.T.astype(bf))
    mats = [adj, proj_eh_h, proj_eh_e, proj_he_h, proj_he_e]

    shared = dict(wqT=np.ascontiguousarray(wqT.astype(bf)),
                  wkT=np.ascontiguousarray(wkT.astype(bf)),
                  wvT=np.ascontiguousarray(wvT.astype(bf)),
                  w1hT=w1hT, w2hT=w2hT, w1eT=w1eT, w2eT=w2eT,
                  b1h=b1_h.astype(f), b2h=b2_h.astype(f),
                  b1e=b1_e.astype(f), b2e=b2_e.astype(f))
    in_maps = []
    for c in range(NCORES):
        r0 = c * RPC
        m = dict(shared)
        m["xn"] = np.ascontiguousarray(np.roll(x_node, -r0, axis=0).astype(bf))
        m["xe"] = np.ascontiguousarray(np.roll(x_edge, -r0, axis=0).astype(bf))
        for i, mat in enumerate(mats):
            mt = mat[r0:r0 + RPC].T.astype(bf)  # [N, RPC]
            m[f"mat{i}"] = np.ascontiguousarray(np.roll(mt, -r0, axis=0))
        in_maps.append(m)
    return in_maps


def kernel(**inputs):
    from concourse.bass_utils import run_bass_kernel_spmd
    nc = _get_program()
    in_maps = _prep_inputs(**{k: np.asarray(v) for k, v in inputs.items()})
    res = run_bass_kernel_spmd(nc, in_maps, list(range(NCORES))).results
    x_h = np.concatenate([res[c]["outh"] for c in range(NCORES)], axis=0)
    x_e = np.concatenate([res[c]["oute"] for c in range(NCORES)], axis=0)
    return (x_h, x_e)



# revision 60
# speedup vs baseline: 1.1993x; 1.0521x over previous
"""Trainium2 Bass kernel for the gnn_message_passing block (8 NeuronCores).

Strategy (per core c, owning 512 global rows r = c*512..(c+1)*512):
  - Host rotates x_node/x_edge rows by -r0 so the owned rows sit first on
    every core (SPMD: one program, per-core data). All heavy DRAM streams
    (x, the five N x N mats, QKV/FFN weights) are pre-cast to bf16 on the
    host, halving HBM traffic.
  - Associativity: mat @ (x @ W.T) == (mat @ x) @ W.T, so the five big
    N x N aggregations (adj@h shared by modules 0/4, four proj@k inputs)
    are computed ONCE per core as row-blocks (mat[r].T streamed from DRAM
    as the bf16 moving operand; bf16 h/e tiles stationary).
  - rmsnorm weight vectors and the 1/sqrt(D) score scale are folded into
    the projection weights host-side; on-chip rmsnorm: Square+accum on
    ACT, rs = exp(-ln(mean+eps)/2) on ACT (stays in the natural_log_exp
    table set shared with the SDPA exp -> ~4 ACT table loads total), and
    the normalizing multiply on DVE (or ACT for bulk h tiles).
  - Per-node 8-head SDPA on DVE in fp16 with access patterns chosen to
    hit the 2x packed mode (microbenchmarked): products with contiguous
    dst + real-inner-dim srcs; halving trees with strided src but
    contiguous dst; exp/den/recip batched once per module (4 tiles) to
    avoid ACT table thrash; 1/den folded in after the AV sum.
  - q+k projections share one 2-bank PSUM tile and one ACT copy; k/v are
    stored d-major for the product patterns.
  - Emission order keeps each engine's in-order queue stall-free: bulk h
    norms run on ACT behind module 1's copies (agg0 unblocks while
    module-1 SDPA runs on DVE); e norms split around module 0; FFN-h sits
    between modules 7/6 so PE fills their SDPA windows; accumulation adds
    run on Pool (kept otherwise idle - Pool SBUF-port traffic degrades
    DVE 2x modes).
"""
import numpy as np

N = 4096
E = 512
H = 8
D = 64
FF = 2048
P = 128
NCORES = 8
RPC = N // NCORES  # 512 rows per core
NT = N // P        # 32 tiles over all nodes
LT = RPC // P      # 4 local tiles
EPS = float(np.finfo(np.float32).eps)

_PROGRAM_CACHE = {}


def _split_big_waits(nc, mybir):
    """walrus in this toolchain rejects multi-wait instructions; cap at 1
    (2 for EventSemaphore), chaining the excess as EventSemaphores."""
    for f in nc.m.functions:
        for bb in f.blocks:
            insts = list(bb.instructions)
            out = []
            changed = False
            for inst in insts:
                si = inst.sync_info
                waits = list(si.on_wait) if si and si.on_wait else []
                cap = 2 if isinstance(inst, mybir.InstEventSemaphore) else 1
                if len(waits) > cap:
                    extra, keep = waits[:-cap], waits[-cap:]
                    for ci in range(0, len(extra), 2):
                        ev = mybir.InstEventSemaphore(name=f"{inst.name}-evw{ci}")
                        ev.engine = inst.engine
                        ev.sync_info = mybir.SyncInfo(on_wait=extra[ci:ci + 2],
                                                      on_update=[])
                        out.append(ev)
                    si.on_wait = keep
                    changed = True
                out.append(inst)
            if changed:
                bb.instructions[:] = out


def _build_program():
    import concourse.bass as bass
    import concourse.tile as tile
    from concourse import mybir
    from concourse.masks import make_identity
    from contextlib import ExitStack

    f32 = mybir.dt.float32
    f32r = mybir.dt.float32r
    f16 = mybir.dt.float16
    bf16 = mybir.dt.bfloat16
    AF = mybir.ActivationFunctionType
    OP = mybir.AluOpType
    AX = mybir.AxisListType

    def bc(t, dims, off=0):
        return bass.AP(tensor=t.tensor, offset=t.offset + off,
                       ap=[list(t.ap[0])] + [[s, c] for (s, c) in dims])

    nc = bass.Bass()

    xn_d = nc.declare_dram_parameter("xn", [N, E], bf16, isOutput=False)
    xe_d = nc.declare_dram_parameter("xe", [N, E], bf16, isOutput=False)
    mat_d = [nc.declare_dram_parameter(f"mat{i}", [N, RPC], bf16, isOutput=False)
             for i in range(5)]
    wq_d = nc.declare_dram_parameter("wqT", [H, E, E], bf16, isOutput=False)
    wk_d = nc.declare_dram_parameter("wkT", [H, E, E], bf16, isOutput=False)
    wv_d = nc.declare_dram_parameter("wvT", [H, E, E], bf16, isOutput=False)
    w1h_d = nc.declare_dram_parameter("w1hT", [E, FF], bf16, isOutput=False)
    w2h_d = nc.declare_dram_parameter("w2hT", [FF, E], bf16, isOutput=False)
    w1e_d = nc.declare_dram_parameter("w1eT", [E, FF], bf16, isOutput=False)
    w2e_d = nc.declare_dram_parameter("w2eT", [FF, E], bf16, isOutput=False)
    b1h_d = nc.declare_dram_parameter("b1h", [FF], f32, isOutput=False)
    b2h_d = nc.declare_dram_parameter("b2h", [E], f32, isOutput=False)
    b1e_d = nc.declare_dram_parameter("b1e", [FF], f32, isOutput=False)
    b2e_d = nc.declare_dram_parameter("b2e", [E], f32, isOutput=False)
    outh_d = nc.declare_dram_parameter("outh", [RPC, E], f32, isOutput=True)
    oute_d = nc.declare_dram_parameter("oute", [RPC, E], f32, isOutput=True)

    with tile.TileContext(nc, pool_alloc_mode="queue") as tc, ExitStack() as ctx:
        consts = ctx.enter_context(tc.tile_pool(name="consts", bufs=1))
        ident = consts.tile([P, P], f32)
        make_identity(nc, ident)
        ident_bf = consts.tile([P, P], bf16)
        nc.scalar.copy(ident_bf[:], ident[:])
        ones1f = consts.tile([1, P], f32)
        nc.gpsimd.memset(ones1f, 1.0)
        ones1 = consts.tile([1, P], f32r)
        nc.scalar.copy(ones1[:], ones1f[:])
        eps_t = consts.tile([P, 1], f32)
        nc.vector.memset(eps_t, EPS)
        b1h_t = consts.tile([P, FF // P], f32)
        nc.gpsimd.dma_start(out=b1h_t, in_=b1h_d[:].rearrange("(c p) -> p c", p=P))
        b1e_t = consts.tile([P, FF // P], f32)
        nc.gpsimd.dma_start(out=b1e_t, in_=b1e_d[:].rearrange("(c p) -> p c", p=P))
        b2h_t = consts.tile([1, E], f32r)
        nc.gpsimd.dma_start(out=b2h_t, in_=b2h_d[:].rearrange("(a e) -> a e", a=1))
        b2e_t = consts.tile([1, E], f32r)
        nc.gpsimd.dma_start(out=b2e_t, in_=b2e_d[:].rearrange("(a e) -> a e", a=1))

        # whole-program pools
        locp = ctx.enter_context(tc.tile_pool(name="loc", bufs=1))
        attp = ctx.enter_context(tc.tile_pool(name="att", bufs=1))
        statp = ctx.enter_context(tc.tile_pool(name="stat", bufs=4))
        sqscp = ctx.enter_context(tc.tile_pool(name="sqsc", bufs=1))
        wpool = ctx.enter_context(tc.tile_pool(name="wts", bufs=1))
        qkvp = ctx.enter_context(tc.tile_pool(name="qkv", bufs=1))
        tmpp = ctx.enter_context(tc.tile_pool(name="sdtmp", bufs=1))
        smp = ctx.enter_context(tc.tile_pool(name="sdsm", bufs=2))
        psp = ctx.enter_context(tc.tile_pool(name="ps", bufs=1, space="PSUM"))

        hTl = [locp.tile([P, RPC], bf16, tag=f"hTl{fc}", name=f"hTl{fc}")
               for fc in range(4)]
        eTl = [locp.tile([P, RPC], bf16, tag=f"eTl{fc}", name=f"eTl{fc}")
               for fc in range(4)]
        xatt_h = [attp.tile([P, E], f32, tag=f"xh{t}", name=f"xh{t}")
                  for t in range(LT)]
        xatt_e = [attp.tile([P, E], f32, tag=f"xe{t}", name=f"xe{t}")
                  for t in range(LT)]

        def load_norm(x_dram, dst_tiles, t0, t1, xpool, on_act=False,
                      on_dve_stt=False):
            """Stream x rows [t0*128, t1*128) in 4-tile DMAs; rmsnorm each.
            ACT computes sum-of-squares + Rsqrt; the normalizing multiply
            runs on DVE (fast path feeding transposes) or fully on ACT
            (bulk tiles, keeping the DVE queue free for SDPA)."""
            for g0 in range(t0, t1, 4):
                ng = min(4, t1 - g0)
                xg = xpool.tile([P, ng * E], bf16, tag="xing", name="xing")
                nc.sync.dma_start(
                    out=xg.rearrange("p (t e) -> p t e", e=E),
                    in_=x_dram[g0 * P:(g0 + ng) * P, :].rearrange(
                        "(t p) e -> p t e", p=P))
                ssq = statp.tile([P, 4], f32, tag="ssq", name="ssq")
                if on_dve_stt:
                    # sum-of-squares on DVE: decoupled from ACT's copy queue
                    sc = sqscp.tile([P, E], bf16, tag="sqscd", name="sqscd")
                    for t in range(ng):
                        nc.vector.scalar_tensor_tensor(
                            out=sc[:], in0=xg[:, t * E:(t + 1) * E], scalar=1.0,
                            in1=xg[:, t * E:(t + 1) * E],
                            op0=OP.mult, op1=OP.mult,
                            accum_out=ssq[:, t:t + 1])
                else:
                    sc = sqscp.tile([P, E], f32, tag="sqsc", name="sqsc")
                    for t in range(ng):
                        nc.scalar.activation(out=sc[:],
                                             in_=xg[:, t * E:(t + 1) * E],
                                             func=AF.Square,
                                             accum_out=ssq[:, t:t + 1])
                # rs = (mean+eps)^-1/2 = exp(-ln(mean+eps)/2): stays in the
                # natural_log_exp ACT table set shared with the SDPA exp
                lnv = statp.tile([P, 4], f32, tag="lnv", name="lnv")
                nc.scalar.activation(out=lnv[:, :ng], in_=ssq[:, :ng],
                                     func=AF.Ln, bias=eps_t[:], scale=1.0 / E)
                rs = statp.tile([P, 4], f32, tag="rs", name="rs")
                nc.scalar.activation(out=rs[:, :ng], in_=lnv[:, :ng],
                                     func=AF.Exp, scale=-0.5)
                for t in range(ng):
                    ti = g0 + t
                    if on_act:
                        nc.scalar.activation(out=dst_tiles[ti][:],
                                             in_=xg[:, t * E:(t + 1) * E],
                                             func=AF.Copy, scale=rs[:, t:t + 1])
                    else:
                        nc.vector.tensor_scalar_mul(
                            dst_tiles[ti][:], xg[:, t * E:(t + 1) * E],
                            rs[:, t:t + 1])

        def rmsnorm_tile(dst, src_ap):
            """dst = pure rmsnorm of node-major [128, 512] slice (FFN path)."""
            sc = sqscp.tile([P, E], f32, tag="sqsc", name="sqsc")
            ssq = statp.tile([P, 1], f32, tag="ssq2", name="ssq2")
            nc.scalar.activation(out=sc[:], in_=src_ap, func=AF.Square,
                                 accum_out=ssq[:])
            lnv = statp.tile([P, 1], f32, tag="lnv2", name="lnv2")
            nc.scalar.activation(out=lnv[:], in_=ssq[:], func=AF.Ln,
                                 bias=eps_t[:], scale=1.0 / E)
            rs = statp.tile([P, 1], f32, tag="rs2", name="rs2")
            nc.scalar.activation(out=rs[:], in_=lnv[:], func=AF.Exp, scale=-0.5)
            nc.vector.tensor_scalar_mul(dst[:], src_ap, rs[:])

        def transpose_local(srcs, dstT):
            for fc in range(4):
                ps = psp.tile([P, RPC], bf16, tag="projps", bufs=2, name="trps")
                for t in range(4):
                    nc.tensor.transpose(ps[:, t * P:(t + 1) * P],
                                        srcs[t][:, fc * P:(fc + 1) * P],
                                        ident_bf[:])
                nc.vector.tensor_copy(dstT[fc][:], ps[:])

        def aggregate(mi, lhs_tiles, aggpool):
            """returns 4 feature-major bf16 [128, 512] blocks of mat_mi @ x."""
            mst = ExitStack()
            matgp = mst.enter_context(tc.tile_pool(name=f"matg{mi}", bufs=2))
            pss = [psp.tile([P, E], f32, tag=f"agps{b}", name=f"agps{b}")
                   for b in range(4)]
            for g in range(8):
                mt = matgp.tile([P, 4 * RPC], bf16, tag="matg", name="matg")
                nc.sync.dma_start(
                    out=mt.rearrange("p (t e) -> p t e", e=RPC),
                    in_=mat_d[mi][g * 4 * P:(g + 1) * 4 * P, :].rearrange(
                        "(t p) e -> p t e", p=P))
                for t in range(4):
                    ti = g * 4 + t
                    for b in range(4):
                        nc.tensor.matmul(
                            pss[b][:],
                            lhsT=lhs_tiles[ti][:, b * P:(b + 1) * P],
                            rhs=mt[:, t * RPC:(t + 1) * RPC],
                            start=(ti == 0), stop=(ti == NT - 1))
            outt = []
            for b in range(4):
                at = aggpool.tile([P, E], bf16, tag=f"ag{mi}_{b}",
                                  name=f"ag{mi}_{b}")
                nc.scalar.copy(at[:], pss[b][:])
                outt.append(at)
            mst.close()
            return outt

        def module(m, qsrc, ksrc, branch_att, first):
            w_ts = {}
            for (dram, nm) in ((wq_d, "wq"), (wk_d, "wk"), (wv_d, "wv")):
                wt = wpool.tile([P, 4 * E], bf16, tag=nm, name=f"w_{nm}")
                nc.sync.dma_start(
                    out=wt.rearrange("p (fc e) -> p fc e", e=E),
                    in_=dram[m].rearrange("(fc p) e -> p fc e", p=P))
                w_ts[nm] = wt

            # phase 1: per tile, project q/k/v and reduce scores into s_all
            s_all = smp.tile([P, LT * H * H], f16, tag="s", name="s")
            v_ts = []
            for t in range(LT):
                # q and k share one 2-bank psum tile and one ACT copy
                qk_ps = psp.tile([P, 2 * E], f32, tag="projqk", bufs=1,
                                 name="qkps")
                for (srcT, wnm, half) in ((qsrc, "wq", 0), (ksrc, "wk", 1)):
                    wt = w_ts[wnm]
                    for fc in range(4):
                        nc.tensor.matmul(
                            qk_ps[:, half * E:(half + 1) * E],
                            lhsT=srcT[fc][:, t * P:(t + 1) * P],
                            rhs=wt[:, fc * E:(fc + 1) * E],
                            start=(fc == 0), stop=(fc == 3))
                qk_t = qkvp.tile([P, 2 * E], f16, tag=f"qk_{t}", name=f"qk{t}")
                nc.scalar.copy(qk_t[:], qk_ps[:])
                q_t, k_t = qk_t, None  # k read via off=E on qk_t

                ps = psp.tile([P, E], f32, tag="projps", bufs=2, name="projps")
                wt = w_ts["wv"]
                for fc in range(4):
                    nc.tensor.matmul(
                        ps[:],
                        lhsT=hTl[fc][:, t * P:(t + 1) * P],
                        rhs=wt[:, fc * E:(fc + 1) * E],
                        start=(fc == 0), stop=(fc == 3))
                dt = qkvp.tile([P, E], f16, tag=f"wv_{t}", name=f"v{t}")
                # stored d-major, head-minor: dt[d*8+g] = ps[g*64+d]
                nc.scalar.copy(bc(dt, [(1, 8), (8, 64)]), ps[:])
                v_ts.append(dt)
                # scores, (h,g,d) layout: tmp[h*512+g*64+d] = q[h,d]*k[g,d]
                tmp = tmpp.tile([P, H * H * D], f16, tag="sdpa", bufs=2,
                                name="sdpa")
                nc.vector.tensor_tensor(
                    out=bc(tmp, [(512, 8), (64, 8), (1, 64)]),
                    in0=bc(q_t, [(64, 8), (0, 8), (1, 64)]),
                    in1=bc(q_t, [(0, 8), (64, 8), (1, 64)], off=E),
                    op=OP.mult)
                # halving tree over inner d: strided src, contiguous dst
                szs = (2048, 1024)
                bufs = [tmpp.tile([P, n], f16, tag=f"pp{n}", bufs=1,
                                  name=f"str{n}") for n in szs]
                src, run = tmp, 32
                for bi, n in enumerate(szs[:2]):
                    nc.vector.tensor_tensor(
                        out=bc(bufs[bi], [(1, n)]),
                        in0=bc(src, [(2 * run, 64), (1, run)]),
                        in1=bc(src, [(2 * run, 64), (1, run)], off=run),
                        op=OP.add)
                    src, run = bufs[bi], run // 2
                # remaining 16-wide groups in one 1x tensor_reduce
                with nc.allow_low_precision(reason="f16 score partial sums"):
                    nc.vector.tensor_reduce(
                        out=bc(s_all, [(1, 64)], off=t * H * H),
                        in_=bc(src, [(16, 64), (1, 16)]),
                        axis=AX.X, op=OP.add)

            # phase 2: one exp / den / recip for all 4 tiles
            ex_all = smp.tile([P, LT * H * H], f16, tag="ex", name="ex")
            nc.scalar.activation(out=ex_all[:], in_=s_all[:], func=AF.Exp)
            den = smp.tile([P, LT * H], f32, tag="den", name="den")
            nc.vector.tensor_reduce(
                out=den[:], in_=ex_all.rearrange("p (th g) -> p th g", g=H),
                axis=AX.X, op=OP.add)
            rden = smp.tile([P, LT * H], f16, tag="rden", name="rden")
            with nc.allow_low_precision(reason="f16 softmax denominators"):
                nc.vector.reciprocal(out=rden[:], in_=den[:])
            # normalize ex once for all 4 tiles: a[t,h,g] = ex[t,h,g]/den[t,h]
            a_all = smp.tile([P, LT * H * H], f16, tag="aall", name="aall")
            nc.vector.tensor_tensor(
                out=a_all[:], in0=ex_all[:],
                in1=bc(rden, [(8, LT), (1, 8), (0, 8)]),
                op=OP.mult)

            # phase 3: AV per tile with unnormalized ex; (h,d,g), g innermost
            for t in range(LT):
                v_t = v_ts[t]
                tmp2 = tmpp.tile([P, H * H * D], f16, tag="sdpa", bufs=2,
                                 name="sdpa2")
                nc.vector.tensor_tensor(
                    out=bc(tmp2, [(512, 8), (8, 64), (1, 8)]),
                    in0=bc(a_all, [(8, 8), (0, 64), (1, 8)], off=t * H * H),
                    in1=bc(v_t, [(0, 8), (8, 64), (1, 8)]),
                    op=OP.mult)
                # pair tree over g: +4 (runs-4), +2 (runs-2), +1 (stride-2)
                av1 = tmpp.tile([P, 2048], f16, tag="pp2048", bufs=1, name="av1")
                nc.vector.tensor_tensor(
                    out=bc(av1, [(1, 2048)]),
                    in0=bc(tmp2, [(8, 512), (1, 4)]),
                    in1=bc(tmp2, [(8, 512), (1, 4)], off=4),
                    op=OP.add)
                av2 = tmpp.tile([P, 1024], f16, tag="pp1024", bufs=1, name="av2")
                nc.vector.tensor_tensor(
                    out=bc(av2, [(1, 1024)]),
                    in0=bc(av1, [(4, 512), (1, 2)]),
                    in1=bc(av1, [(4, 512), (1, 2)], off=2),
                    op=OP.add)
                # last pair-sum writes the branch accumulator directly
                if first:
                    nc.vector.tensor_tensor(
                        out=branch_att[t][:],
                        in0=bc(av2, [(2, 512)]),
                        in1=bc(av2, [(2, 512)], off=1),
                        op=OP.add)
                else:
                    rt = smp.tile([P, E], f16, tag="avred", name="avred")
                    nc.vector.tensor_tensor(
                        out=rt[:],
                        in0=bc(av2, [(2, 512)]),
                        in1=bc(av2, [(2, 512)], off=1),
                        op=OP.add)
                    nc.gpsimd.tensor_tensor(out=branch_att[t][:],
                                            in0=branch_att[t][:], in1=rt[:],
                                            op=OP.add)

        def ffn(branch_att, w1_dram, w2_dram, b1_t, b2_t, out_dram):
            with tc.tile_pool(name="ffn_sb", bufs=1) as fsb, \
                 tc.tile_pool(name="ffn_xn", bufs=1) as fxn:
                xn_tiles = []
                for t in range(LT):
                    xt = fxn.tile([P, E], bf16, tag=f"fx{t}", name=f"fx{t}")
                    rmsnorm_tile(xt, branch_att[t][:])
                    xn_tiles.append(xt)
                xnT = []
                for fc in range(4):
                    ps = psp.tile([P, RPC], bf16, tag="agps0", name="ftr")
                    for t in range(4):
                        nc.tensor.transpose(ps[:, t * P:(t + 1) * P],
                                            xn_tiles[t][:, fc * P:(fc + 1) * P],
                                            ident_bf[:])
                    xt = fxn.tile([P, RPC], bf16, tag=f"fxT{fc}", name=f"fxT{fc}")
                    nc.scalar.copy(xt[:], ps[:])
                    xnT.append(xt)
                g1 = []
                HW1 = FF // 2
                for half in range(2):
                    w1_t = fsb.tile([P, 4 * HW1], bf16, tag="w1", name="w1")
                    nc.sync.dma_start(
                        out=w1_t.rearrange("p (fc e) -> p fc e", e=HW1),
                        in_=w1_dram[:, half * HW1:(half + 1) * HW1].rearrange(
                            "(fc p) e -> p fc e", p=P))
                    for fb in range(HW1 // P):
                        ffb = half * (HW1 // P) + fb
                        ps = psp.tile([P, RPC], f32, tag=f"agps{1 + ffb % 2}",
                                      name="fps1")
                        for fc in range(4):
                            nc.tensor.matmul(
                                ps[:],
                                lhsT=w1_t[:, fc * HW1 + fb * P:
                                          fc * HW1 + (fb + 1) * P],
                                rhs=xnT[fc][:],
                                start=(fc == 0), stop=(fc == 3))
                        gt = fsb.tile([P, RPC], bf16, tag=f"g1_{ffb}",
                                      name=f"g1_{ffb}")
                        nc.scalar.activation(out=gt[:], in_=ps[:], func=AF.Gelu,
                                             bias=b1_t[:, ffb:ffb + 1], scale=1.0)
                        g1.append(gt)
                w2_t = fsb.tile([P, 16 * E], bf16, tag="w2", name="w2")
                nc.sync.dma_start(
                    out=w2_t.rearrange("p (fc e) -> p fc e", e=E),
                    in_=w2_dram[:, :].rearrange("(fc p) e -> p fc e", p=P))
                ot = fsb.tile([P, 4 * E], f32, tag="fo", name="fo")
                for b in range(LT):
                    ps = psp.tile([P, E], f32, tag="agps3", name="fps2")
                    for ffc in range(FF // P):
                        nc.tensor.matmul(
                            ps[:],
                            lhsT=g1[ffc][:, b * P:(b + 1) * P],
                            rhs=w2_t[:, ffc * E:(ffc + 1) * E],
                            start=(ffc == 0), stop=False)
                    nc.tensor.matmul(ps[:], lhsT=ones1[:], rhs=b2_t[:],
                                     start=False, stop=True)
                    nc.scalar.copy(ot[:, b * E:(b + 1) * E], ps[:])
                nc.sync.dma_start(
                    out=out_dram[:, :].rearrange("(b p) e -> p b e", p=P),
                    in_=ot.rearrange("p (b e) -> p b e", e=E))

        # ======== emission order (the schedule) ========
        # Pool open/close must be LIFO: agglate (aggs 1-4) and eearly
        # outlive hfull; agg0/erest nest inside.
        aggl12_stack = ExitStack()
        agglate12 = aggl12_stack.enter_context(
            tc.tile_pool(name="agglate12", bufs=1))
        aggl34_stack = ExitStack()
        agglate34 = aggl34_stack.enter_context(
            tc.tile_pool(name="agglate34", bufs=1))
        eearly_stack = ExitStack()
        eearly = eearly_stack.enter_context(tc.tile_pool(name="eearly", bufs=1))
        hstack = ExitStack()
        hfp = hstack.enter_context(tc.tile_pool(name="hfull", bufs=1))

        h_sb = [hfp.tile([P, E], bf16, tag=f"h{t}", name=f"hsb{t}")
                for t in range(NT)]
        e_sb = [eearly.tile([P, E], bf16, tag=f"e{t}", name=f"esb{t}")
                for t in range(NT)]

        load_norm(xn_d, h_sb, 0, 4, hfp)
        load_norm(xe_d, e_sb, 0, 4, eearly)
        transpose_local(h_sb[:4], hTl)
        transpose_local(e_sb[:4], eTl)

        # module 1 needs no aggregate (only hTl/eTl) - start DVE early
        module(1, eTl, eTl, xatt_h, first=True)

        # bulk h norms run fully on ACT (after module 1's copies in the ACT
        # queue) so agg0's matmul groups unblock progressively while module
        # 1's SDPA runs on DVE
        load_norm(xn_d, h_sb, 4, NT, hfp, on_act=True)

        agg0_stack = ExitStack()
        agg0pool = agg0_stack.enter_context(tc.tile_pool(name="agg0p", bufs=1))
        agg0 = aggregate(0, h_sb, agg0pool)

        module(5, eTl, eTl, xatt_e, first=True)
        # e bulk with DVE sum-of-squares so the norm chain doesn't queue
        # behind copies on ACT; split around module 0
        load_norm(xe_d, e_sb, 4, 16, eearly, on_dve_stt=True)

        agg1 = aggregate(1, h_sb, agglate12)
        agg2 = aggregate(2, h_sb, agglate12)

        module(0, agg0, hTl, xatt_h, first=False)
        load_norm(xe_d, e_sb, 16, NT, eearly, on_dve_stt=True)
        module(4, agg0, hTl, xatt_e, first=False)
        agg0_stack.close()
        hstack.close()

        agg3 = aggregate(3, e_sb, agglate34)
        agg4 = aggregate(4, e_sb, agglate34)

        module(2, eTl, agg1, xatt_h, first=False)
        module(3, hTl, agg3, xatt_h, first=False)
        eearly_stack.close()
        module(7, hTl, agg4, xatt_e, first=False)
        aggl34_stack.close()
        ffn(xatt_h, w1h_d, w2h_d, b1h_t, b2h_t, outh_d)
        module(6, eTl, agg2, xatt_e, first=False)
        aggl12_stack.close()
        ffn(xatt_e, w1e_d, w2e_d, b1e_t, b2e_t, oute_d)

    _split_big_waits(nc, mybir)
    return nc


def _get_program():
    if "nc" not in _PROGRAM_CACHE:
        _PROGRAM_CACHE["nc"] = _build_program()
    return _PROGRAM_CACHE["nc"]


def _prep_inputs(x_node, x_edge, adj, Wq, Wk, Wv,
                 proj_he_h, proj_eh_h, proj_he_e, proj_eh_e,
                 rms1_h, rms1_e, rms2_h,
                 w1_h, b1_h, w2_h, b2_h, w1_e, b1_e, w2_e, b2_e):
    """Per-core input dicts. Weight folding + row rotation happen here."""
    import ml_dtypes
    f = np.float32
    bf = ml_dtypes.bfloat16
    wsrc_q = [rms1_h, rms1_e, rms1_e, rms1_h, rms1_h, rms1_e, rms1_e, rms1_h]
    wsrc_k = [rms1_h, rms1_e, rms1_h, rms1_e, rms1_h, rms1_e, rms1_h, rms1_e]
    wqT = np.stack([(Wq[m].T * wsrc_q[m][:, None]) * 0.125 for m in range(H)])
    wkT = np.stack([Wk[m].T * wsrc_k[m][:, None] for m in range(H)])
    wvT = np.stack([Wv[m].T * rms1_h[:, None] for m in range(H)])
    w1hT = np.ascontiguousarray((w1_h * rms2_h[None, :]).T.astype(bf))
    w1eT = np.ascontiguousarray((w1_e * rms2_h[None, :]).T.astype(bf))
    w2hT = np.ascontiguousarray(w2_h.T.astype(bf))
    w2eT = np.ascontiguousarray(w2_e.T.astype(bf))
    mats = [adj, proj_eh_h, proj_eh_e, proj_he_h, proj_he_e]

    shared = dict(wqT=np.ascontiguousarray(wqT.astype(bf)),
                  wkT=np.ascontiguousarray(wkT.astype(bf)),
                  wvT=np.ascontiguousarray(wvT.astype(bf)),
                  w1hT=w1hT, w2hT=w2hT, w1eT=w1eT, w2eT=w2eT,
                  b1h=b1_h.astype(f), b2h=b2_h.astype(f),
                  b1e=b1_e.astype(f), b2e=b2_e.astype(f))
    in_maps = []
    for c in range(NCORES):
        r0 = c * RPC
        m = dict(shared)
        m["xn"] = np.ascontiguousarray(np.roll(x_node, -r0, axis=0).astype(bf))
        m["xe"] = np.ascontiguousarray(np.roll(x_edge, -r0, axis=0).astype(bf))
        for i, mat in enumerate(mats):
            mt = mat[r0:r0 + RPC].T.astype(bf)  # [N, RPC]
            m[f"mat{i}"] = np.ascontiguousarray(np.roll(mt, -r0, axis=0))
        in_maps.append(m)
    return in_maps


def kernel(**inputs):
    from concourse.bass_utils import run_bass_kernel_spmd
    nc = _get_program()
    in_maps = _prep_inputs(**{k: np.asarray(v) for k, v in inputs.items()})
    res = run_bass_kernel_spmd(nc, in_maps, list(range(NCORES))).results
    x_h = np.concatenate([res[c]["outh"] for c in range(NCORES)], axis=0)
    x_e = np.concatenate([res[c]["oute"] for c in range(NCORES)], axis=0)
    return (x_h, x_e)

